# revision 1
# baseline (speedup 1.0000x reference)
"""Trainium2 Bass kernel for causal softmax-free multi-head attention (retention).

Reference computation (per batch b):
    kqv = x @ W1 + b1 ; k, q, v = split(kqv, 3)   [split order k, q, v]
    per head h (dh = 64):  attn = tril(q_h @ k_h^T) ; o_h = attn @ v_h
    out = concat_h(o_h) @ W2 + b2

Sharding: 8 cores = 2 batches x 4 head-groups (4 heads each). Each core
computes its batch's projections restricted to its heads' weight columns,
the attention for its 4 heads, and a partial output projection
(out_local @ W2[rows of its heads]). Host sums the 4 partials per batch.

Algorithm: chunked linear attention. tril(QK^T)V is computed per 256-token
block as  O = Q @ S + tril_block(Q K_blk^T) V_blk, with the running state
S = K^T V accumulated over previous blocks ([64,64] per head). This turns
the O(T^2 dh) dense attention into O(T c dh + T dh^2) work.

Hardware constraints honored (empirically validated on trn2):
  - fp32r matmuls require K=128 contraction, M=128 stationary free dim,
    and N>=256 moving free dim; anything else corrupts results.
    -> dh=64 contractions are zero-padded to 128 rows (kTpad, Spad).
    -> M=64 stationary operands are widened to 128 (head pairs / padding),
       producing garbage rows that are simply never read back.
  - DMA cannot touch PSUM: every matmul result is copied out via DVE/ACT.
  - Producers of fp32r matmul operands must write f32r-typed outputs.
"""

import numpy as np

import concourse.bacc as bacc
import concourse.mybir as mybir
import concourse.tile as tile
from concourse.bass_utils import run_bass_kernel_spmd

F32 = mybir.dt.float32
F32R = mybir.dt.float32r
AF = mybir.ActivationFunctionType

B, T, D = 2, 2048, 1024
H, DH = 16, 64
HPC = 4           # heads per core
FH = HPC * DH     # 256 features per core per tensor
BLK = 256         # state-update block (2 x 128-token chunks)
NBLK = T // BLK   # 8
NTC = T // 128    # 16 token chunks
ND = D // 128     # 8 contraction chunks
NQT = T // 512    # 4 wide token tiles

TRACE = False
TRACE_DIR = None
LAST_RESULTS = [None]


def _build():
    nc = bacc.Bacc("TRN2", target_bir_lowering=False, debug=False, num_devices=8)

    xT = nc.dram_tensor("xT", [D, T], F32, kind="ExternalInput").ap()
    w1q = nc.dram_tensor("w1q", [D, FH], F32, kind="ExternalInput").ap()
    w1kv = nc.dram_tensor("w1kv", [D, 2 * FH], F32, kind="ExternalInput").ap()
    b1q = nc.dram_tensor("b1q", [FH], F32, kind="ExternalInput").ap()
    b1kv = nc.dram_tensor("b1kv", [2 * FH], F32, kind="ExternalInput").ap()
    w2 = nc.dram_tensor("w2", [FH, D], F32, kind="ExternalInput").ap()
    mask0 = nc.dram_tensor("mask0", [128, BLK], F32, kind="ExternalInput").ap()
    mask1 = nc.dram_tensor("mask1", [128, BLK], F32, kind="ExternalInput").ap()
    zer = nc.dram_tensor("zer", [128, T], F32, kind="ExternalInput").ap()
    out = nc.dram_tensor("out", [D, T], F32, kind="ExternalOutput").ap()

    r = lambda ap: ap.bitcast(F32R)

    with tile.TileContext(nc) as tc:
        with (
            tc.tile_pool(name="persist", bufs=1) as pp,
            tc.tile_pool(name="work", bufs=3) as wp,
            tc.tile_pool(name="psA", bufs=4, space="PSUM") as psA,
            tc.tile_pool(name="psO", bufs=2, space="PSUM") as psO,
            tc.tile_pool(name="psU", bufs=2, space="PSUM") as psU,
        ):
            # ---- persistent SBUF tiles -------------------------------------
            w1q_sb = pp.tile([128, ND * FH], F32, name="w1q_sb", tag="w1q_sb")
            w1kv_sb = pp.tile([128, ND * 2 * FH], F32, name="w1kv_sb", tag="w1kv_sb")
            b1q_sb = pp.tile([128, 2], F32, name="b1q_sb", tag="b1q_sb")
            b1k_sb = pp.tile([128, 2], F32, name="b1k_sb", tag="b1k_sb")
            bkv_sb = pp.tile([128, 512], F32, name="bkv_sb", tag="bkv_sb")
            m0_sb = pp.tile([128, BLK], F32, name="m0_sb", tag="m0_sb")
            m1_sb = pp.tile([128, BLK], F32, name="m1_sb", tag="m1_sb")
            qT_sb = [pp.tile([128, T], F32, name=f"qT{g}", tag=f"qT{g}") for g in range(2)]
            kTpad = [pp.tile([128, T], F32, name=f"kTpad{h}", tag=f"kTpad{h}") for h in range(4)]
            kv_sb = [pp.tile([128, 512], F32, name=f"kv{t}", tag=f"kv{t}") for t in range(NTC)]
            oT_sb = [pp.tile([128, T], F32, name=f"oT{g}", tag=f"oT{g}") for g in range(2)]

            with tc.tile_pool(name="xt", bufs=1) as xp:
                xt = [xp.tile([128, T], F32, name=f"xt{i}", tag=f"xt{i}") for i in range(ND)]
                # x^T chunks on gpsimd queues, weight chunks on sync queues —
                # DMA triggers cost ~0.65us each on the issuing engine, so
                # spread them and keep the count low.
                # Chunk-interleaved input stream on one queue: the projection
                # waves below consume chunk d of (x^T, W1q, W1kv) together, so
                # deliver them together and in order.
                nc.sync.dma_start(out=r(m0_sb[:]), in_=r(mask0[:]))
                nc.sync.dma_start(out=r(m1_sb[:]), in_=r(mask1[:]))
                # Column-halved x^T stream: all chunks' first 1024 token-cols
                # land first, so the qt0-1 projection waves (and the first 8
                # KV groups) are fully runnable while the second half streams.
                HT = T // 2
                for i in range(ND):
                    nc.sync.dma_start(out=r(xt[i][:, 0:HT]), in_=r(xT[128 * i:128 * (i + 1), 0:HT]))
                    nc.sync.dma_start(
                        out=r(w1q_sb[:, i * FH:(i + 1) * FH]),
                        in_=r(w1q[128 * i:128 * (i + 1), :]))
                    nc.gpsimd.dma_start(
                        out=r(w1kv_sb[:, i * 2 * FH:(i + 1) * 2 * FH]),
                        in_=r(w1kv[128 * i:128 * (i + 1), :]))
                    if i == 2:
                        nc.gpsimd.dma_start(out=b1q_sb[:], in_=b1q.rearrange("(c p) -> p c", p=128))
                        nc.gpsimd.dma_start(out=b1k_sb[:], in_=b1kv[0:256].rearrange("(c p) -> p c", p=128))
                        nc.gpsimd.dma_start(out=bkv_sb[:], in_=b1kv.unsqueeze(0).broadcast_to([128, 512]))
                for i in range(ND):
                    nc.sync.dma_start(out=r(xt[i][:, HT:T]), in_=r(xT[128 * i:128 * (i + 1), HT:T]))
                # zero fills last — only needed by phase C
                for h in range(4):
                    par = h % 2
                    nc.sync.dma_start(
                        out=r(kTpad[h][(1 - par) * 64:(2 - par) * 64, :]),
                        in_=r(zer[0:64, :]))

                # ---- phase B: projections ----------------------------------
                # Waves of 8 concurrent PSUM groups; within a wave the
                # contraction chunk d is the OUTER loop so the (in-order) PE
                # stream can run each chunk's matmuls as soon as that chunk
                # lands, instead of blocking on the last chunk of group 0.
                _pools = [(psA, "pa"), (psU, "pu"), (psO, "po"),
                          (psA, "pa"), (psU, "pu"), (psO, "po"),
                          (psA, "pa"), (psA, "pa")]

                def run_wave(groups, pools=None):
                    # groups: list of (lhsT_fn(d), rhs_fn(d), copyback_fn)
                    pl = pools if pools is not None else _pools
                    tiles = []
                    for gi, _ in enumerate(groups):
                        pool, tag = pl[gi]
                        tiles.append(pool.tile([128, 512], F32, name=f"pw{gi}", tag=tag))
                    for d in range(ND):
                        for gi, (lf, rf, _) in enumerate(groups):
                            nc.tensor.matmul(
                                tiles[gi][:], lf(d), rf(d),
                                start=(d == 0), stop=(d == ND - 1))
                    for gi, (_, _, cb) in enumerate(groups):
                        cb(tiles[gi])

                def q_group(ft, qt):
                    def cb(pt):
                        nc.scalar.activation(
                            r(qT_sb[ft][:, qt * 512:(qt + 1) * 512]), pt[:],
                            AF.Identity, bias=b1q_sb[:, ft:ft + 1])
                    return (
                        lambda d: r(w1q_sb[:, d * FH + ft * 128: d * FH + (ft + 1) * 128]),
                        lambda d: r(xt[d][:, qt * 512:(qt + 1) * 512]),
                        cb)

                def k_group(ft, qt):
                    def cb(pt):
                        for par in range(2):
                            h = 2 * ft + par
                            sl = slice(par * 64, (par + 1) * 64)
                            nc.scalar.activation(
                                r(kTpad[h][sl, qt * 512:(qt + 1) * 512]), pt[sl, :],
                                AF.Identity, bias=b1k_sb[sl, ft:ft + 1])
                    return (
                        lambda d: r(w1kv_sb[:, d * 2 * FH + ft * 128: d * 2 * FH + (ft + 1) * 128]),
                        lambda d: r(xt[d][:, qt * 512:(qt + 1) * 512]),
                        cb)

                def kv_group(tcn):
                    def cb(pt):
                        nc.vector.tensor_tensor(
                            r(kv_sb[tcn][:]), pt[:], bkv_sb[:], mybir.AluOpType.add)
                    return (
                        lambda d: r(xt[d][:, tcn * 128:(tcn + 1) * 128]),
                        lambda d: r(w1kv_sb[:, d * 2 * FH:(d + 1) * 2 * FH]),
                        cb)

                # Wave order matched to the half-column stream: W1 (qt0-1
                # of Q^T/K^T) runs during the first half, W3 (KV tcn0-7, all
                # first-half data) keeps the PE saturated while the second
                # half streams, then W2 (qt2-3) and W4 (KV tcn8-15).
                run_wave([q_group(0, 0), q_group(0, 1), k_group(0, 0), k_group(0, 1),
                          q_group(1, 0), q_group(1, 1), k_group(1, 0), k_group(1, 1)])
                run_wave([kv_group(t) for t in range(8)])
                run_wave([q_group(0, 2), q_group(0, 3), k_group(0, 2), k_group(0, 3),
                          q_group(1, 2), q_group(1, 3), k_group(1, 2), k_group(1, 3)])
                run_wave([kv_group(t) for t in range(8, 16)])

            # ---- late pool: state tiles + W2 (reuses x^T space) ------------
            with tc.tile_pool(name="late", bufs=1) as lp:
                spad = [lp.tile([128, 128], F32, name=f"spad{h}", tag=f"spad{h}") for h in range(4)]
                w2_sb = lp.tile([128, 2 * D], F32, name="w2_sb", tag="w2_sb")
                # manually-rotated a1 ring: the left half of each slot is the
                # always-zero region of the chunk-1 scores; zero it once via
                # DMA and let the per-block mask multiply touch only the tril
                # half. Tile tracks WAR deps on the persistent tiles.
                a1ring = [lp.tile([128, 2 * BLK], F32, name=f"a1r{i}", tag=f"a1r{i}")
                          for i in range(4)]
                for i in range(4):
                    for par in range(2):
                        nc.gpsimd.dma_start(
                            out=r(a1ring[i][:, par * BLK: par * BLK + 128]),
                            in_=r(zer[:, 0:128]))
                for h in range(4):
                    nc.gpsimd.dma_start(out=r(spad[h][:]), in_=r(zer[:, 0:128]))
                nc.sync.dma_start(
                    out=r(w2_sb.rearrange("p (c f) -> p c f", c=2)),
                    in_=r(w2.rearrange("(c p) f -> p c f", p=128)))

                # ---- phase C: chunked causal attention + interleaved D -----
                # Two-stage software pipeline: block m's scores are emitted
                # before block m-1's O-accumulation chains, so the in-order PE
                # stream always has independent matmuls while the DVE applies
                # causal masks for the previous block.
                ablk = {}

                def scores_block(m):
                    qsl = slice(m * BLK, (m + 1) * BLK)
                    for pg in range(2):
                        a0 = lp.tile([128, 2 * BLK], F32, name="a0", tag="a0", bufs=4)
                        a1 = a1ring[(2 * m + pg) % 4]
                        ablk[(m, pg)] = (a0, a1)
                        for par in range(2):
                            h = 2 * pg + par
                            asl = slice(par * BLK, (par + 1) * BLK)
                            pA = psA.tile([128, 2 * BLK], F32, name="pA", tag="pa")
                            nc.tensor.matmul(
                                pA[:, 0:BLK], r(kTpad[h][:, (2 * m) * 128:(2 * m + 1) * 128]),
                                r(qT_sb[pg][:, qsl]), start=True, stop=True)
                            nc.tensor.matmul(
                                pA[:, BLK:2 * BLK], r(kTpad[h][:, (2 * m + 1) * 128:(2 * m + 2) * 128]),
                                r(qT_sb[pg][:, qsl]), start=True, stop=True, skip_group_check=True)
                            nc.vector.tensor_tensor(r(a0[:, asl]), pA[:, 0:BLK], m0_sb[:], mybir.AluOpType.mult)
                            # only the tril half: the left 128 cols stay zero
                            nc.vector.tensor_tensor(
                                r(a1[:, par * BLK + 128: (par + 1) * BLK]),
                                pA[:, BLK + 128:2 * BLK], m1_sb[:, 128:BLK],
                                mybir.AluOpType.mult)

                def chains_block(m):
                    qsl = slice(m * BLK, (m + 1) * BLK)
                    for pg in range(2):
                        a0, a1 = ablk.pop((m, pg))
                        pO = psO.tile([128, 2 * BLK], F32, name="pO", tag="po")
                        nc.tensor.matmul(
                            pO[:], r(kv_sb[2 * m][:, FH + pg * 128: FH + (pg + 1) * 128]),
                            r(a0[:]), start=True, stop=False)
                        nc.tensor.matmul(
                            pO[:], r(kv_sb[2 * m + 1][:, FH + pg * 128: FH + (pg + 1) * 128]),
                            r(a1[:]), start=False, stop=(m == 0))
                        if m > 0:
                            nc.tensor.matmul(
                                pO[:, 0:BLK], r(spad[2 * pg][:]), r(qT_sb[pg][:, qsl]),
                                start=False, stop=False)
                            nc.tensor.matmul(
                                pO[:, BLK:2 * BLK], r(spad[2 * pg + 1][:]), r(qT_sb[pg][:, qsl]),
                                start=False, stop=True)
                        for par in range(2):
                            hr = slice(par * 64, (par + 1) * 64)
                            nc.scalar.activation(
                                r(oT_sb[pg][hr, qsl]),
                                pO[hr, par * BLK:par * BLK + BLK], AF.Identity)

                    for pg in range(2):
                        pU = psU.tile([128, BLK], F32, name="pU", tag="pu")
                        nc.tensor.matmul(
                            pU[:], r(kv_sb[2 * m][:, pg * 128:(pg + 1) * 128]),
                            r(kv_sb[2 * m][:, FH:2 * FH]), start=True, stop=False)
                        nc.tensor.matmul(
                            pU[:], r(kv_sb[2 * m + 1][:, pg * 128:(pg + 1) * 128]),
                            r(kv_sb[2 * m + 1][:, FH:2 * FH]), start=False, stop=True)
                        for par in range(2):
                            h = 2 * pg + par
                            hr = slice(par * 64, (par + 1) * 64)
                            nc.vector.tensor_tensor(
                                r(spad[h][hr, hr]), pU[hr, h * 64:(h + 1) * 64],
                                spad[h][hr, hr], mybir.AluOpType.add)

                def proj_tile(qt, half):
                    dcr = range(0, ND // 2) if half == 0 else range(ND // 2, ND)
                    for dc in dcr:
                        pf = [psA.tile([128, 512], F32, name="pf", tag="pa"),
                              psU.tile([128, 512], F32, name="pf2", tag="pu"),
                              psO.tile([128, 512], F32, name="pf3", tag="po")][dc % 3]
                        for g2 in range(2):
                            nc.tensor.matmul(
                                pf[:],
                                r(w2_sb[:, g2 * D + dc * 128: g2 * D + (dc + 1) * 128]),
                                r(oT_sb[g2][:, qt * 512:(qt + 1) * 512]),
                                start=(g2 == 0), stop=(g2 == 1))
                        fs = lp.tile([128, 512], F32, name="fs", tag="fs", bufs=3)
                        if dc % 2 == 0:
                            nc.vector.tensor_copy(fs[:], pf[:])
                        else:
                            nc.scalar.activation(fs[:], pf[:], AF.Identity)
                        dma_eng = nc.gpsimd if dc % 2 == 0 else nc.sync
                        dma_eng.dma_start(
                            out=out[dc * 128:(dc + 1) * 128, qt * 512:(qt + 1) * 512],
                            in_=fs[:])

                def proj_tile256(tcn):
                    for dc in range(ND):
                        pf = psA.tile([128, 2 * BLK], F32, name="pf3", tag="pa") if dc % 2 == 0 \
                            else psU.tile([128, 2 * BLK], F32, name="pf4", tag="pu")
                        for g2 in range(2):
                            nc.tensor.matmul(
                                pf[:, 0:BLK],
                                r(w2_sb[:, g2 * D + dc * 128: g2 * D + (dc + 1) * 128]),
                                r(oT_sb[g2][:, tcn * BLK:(tcn + 1) * BLK]),
                                start=(g2 == 0), stop=(g2 == 1))
                        fs = lp.tile([128, BLK], F32, name="fs2", tag="fs2", bufs=3)
                        if dc % 2 == 0:
                            nc.vector.tensor_copy(fs[:], pf[:, 0:BLK])
                        else:
                            nc.scalar.activation(fs[:], pf[:, 0:BLK], AF.Identity)
                        dma_eng = nc.gpsimd if dc % 2 == 0 else nc.sync
                        dma_eng.dma_start(
                            out=out[dc * 128:(dc + 1) * 128, tcn * BLK:(tcn + 1) * BLK],
                            in_=fs[:])

                # proj_tile(qt) is emitted one full block after the ACT
                # copybacks that produce its oT inputs, so the PE stream never
                # stalls waiting for the Scalar engine to catch up.
                # D tiles are spread as half-emissions (4 dout chunks each)
                # across blocks, one-plus blocks after the chains that produce
                # their oT inputs.
                dplan = {3: (0, 0), 4: (0, 1), 5: (1, 0), 6: (1, 1), 7: (2, 0)}
                scores_block(0)
                for m in range(1, NBLK):
                    scores_block(m)
                    chains_block(m - 1)
                    if m in dplan:
                        proj_tile(*dplan[m])
                chains_block(NBLK - 1)
                proj_tile(2, 1)
                proj_tile(3, 0)
                proj_tile(3, 1)

    nc.compile()
    return nc


_NC = None


def _get_nc():
    global _NC
    if _NC is None:
        _NC = _build()
    return _NC


def make_core_inputs(x, W1, b1, W2, b2):
    """Shard full inputs into the 8 per-core input dicts."""
    x = np.asarray(x, dtype=np.float32)
    W1 = np.asarray(W1, dtype=np.float32)
    b1 = np.asarray(b1, dtype=np.float32)
    W2 = np.asarray(W2, dtype=np.float32)

    p = np.arange(128)[:, None]
    f = np.arange(BLK)[None, :]
    mask0 = (f >= p).astype(np.float32)
    mask1 = (f >= p + 128).astype(np.float32)
    zer = np.zeros((128, T), np.float32)

    in_maps = []
    for c in range(8):
        b = c // 4
        g = c % 4
        ksl = slice(g * FH, (g + 1) * FH)
        qsl = slice(D + g * FH, D + (g + 1) * FH)
        vsl = slice(2 * D + g * FH, 2 * D + (g + 1) * FH)
        in_maps.append({
            "xT": np.ascontiguousarray(x[b].T),
            "w1q": np.ascontiguousarray(W1[:, qsl]),
            "w1kv": np.ascontiguousarray(np.concatenate([W1[:, ksl], W1[:, vsl]], axis=1)),
            "b1q": np.ascontiguousarray(b1[qsl]),
            "b1kv": np.ascontiguousarray(np.concatenate([b1[ksl], b1[vsl]])),
            "w2": np.ascontiguousarray(W2[ksl, :]),
            "mask0": mask0,
            "mask1": mask1,
            "zer": zer,
        })
    return in_maps


def kernel(x, W1, b1, W2, b2):
    nc = _get_nc()
    in_maps = make_core_inputs(x, W1, b1, W2, b2)
    kwargs = {}
    if TRACE:
        kwargs = {"trace": True, "tmpdir": TRACE_DIR}
    res = run_bass_kernel_spmd(nc, in_maps, list(range(8)), **kwargs)
    LAST_RESULTS[0] = res
    b2 = np.asarray(b2, dtype=np.float32)
    out = np.zeros((B, T, D), np.float32)
    for c in range(8):
        out[c // 4] += res.results[c]["out"].T
    out += b2[None, None, :]
    return out



# revision 2
# speedup vs baseline: 1.2518x; 1.2518x over previous
"""Trainium2 Bass kernel for causal softmax-free multi-head attention (retention).

Reference computation (per batch b):
    kqv = x @ W1 + b1 ; k, q, v = split(kqv, 3)   [split order k, q, v]
    per head h (dh = 64):  attn = tril(q_h @ k_h^T) ; o_h = attn @ v_h
    out = concat_h(o_h) @ W2 + b2

Sharding: 8 cores = 2 batches x 4 head-groups (4 heads each). Each core
computes its batch's projections restricted to its heads' weight columns,
the attention for its 4 heads, and a partial output projection
(out_local @ W2[rows of its heads]). Host sums the 4 partials per batch.

v2 vs v1:
  - all matmul operands are bf16 (PSUM accumulation stays f32): halves DMA
    traffic in (x/W1/W2 stream as bf16) and out (bf16 partials), and halves
    DVE element time. Max rel err vs f32 reference ~6e-3 (gate 2e-2).
  - K is projected ONCE ([tok, feat] layout inside the KV wave); the
    [feat, tok] layout needed by the score matmuls comes from PE transposes
    (32 x [128,128] via identity), replacing the v1 duplicate K projection
    (32768 PE rows -> 4096).
  - scores select heads by zero-padding Q (qTp[h], memset halves) instead
    of zero-padding K, so the transposed K stays packed.
  - zero fills via engine memset, not DMA (kills the zer input stream).
  - output projection spread earlier across phase C; the last two blocks
    are emitted at 256-token granularity so the tail after the final
    chains is only 2 small proj groups; out-DMAs pair two dout chunks
    per transfer ([128, 2, 256/512] APs) to halve trigger count.

Algorithm: chunked linear attention. tril(QK^T)V is computed per 256-token
block as  O = Q @ S + tril_block(Q K_blk^T) V_blk, with the running state
S = K^T V accumulated over previous blocks ([64,64] per head).

Hardware constraints honored:
  - matmul tiles: K=128 contraction, M=128 stationary, N>=256 moving
    (N=128 only for the PE transposes, which are exact data movement).
  - DMA cannot touch PSUM: every matmul result is copied out via DVE/ACT.
"""

import numpy as np
import ml_dtypes

import concourse.bacc as bacc
import concourse.mybir as mybir
import concourse.tile as tile
from concourse.bass_utils import run_bass_kernel_spmd

F32 = mybir.dt.float32
BF16 = mybir.dt.bfloat16
AF = mybir.ActivationFunctionType
BF = ml_dtypes.bfloat16

B, T, D = 2, 2048, 1024
H, DH = 16, 64
HPC = 4           # heads per core
FH = HPC * DH     # 256 features per core per tensor
BLK = 256         # state-update block (2 x 128-token chunks)
NBLK = T // BLK   # 8
NTC = T // 128    # 16 token chunks
ND = D // 128     # 8 contraction chunks

# set False if mixed-dtype (f32 PSUM + bf16 SBUF) tensor_tensor misbehaves
MIXED_TT = True

TRACE = False
TRACE_DIR = None
LAST_RESULTS = [None]


def _build():
    nc = bacc.Bacc("TRN2", target_bir_lowering=False, debug=False, num_devices=8)

    xT = nc.dram_tensor("xT", [D, T], BF16, kind="ExternalInput").ap()
    w1q = nc.dram_tensor("w1q", [D, FH], BF16, kind="ExternalInput").ap()
    w1kv = nc.dram_tensor("w1kv", [D, 2 * FH], BF16, kind="ExternalInput").ap()
    b1q = nc.dram_tensor("b1q", [FH], F32, kind="ExternalInput").ap()
    b1kv = nc.dram_tensor("b1kv", [2 * FH], F32, kind="ExternalInput").ap()
    w2 = nc.dram_tensor("w2", [FH, D], BF16, kind="ExternalInput").ap()
    mask0 = nc.dram_tensor("mask0", [128, BLK], F32, kind="ExternalInput").ap()
    mask1 = nc.dram_tensor("mask1", [128, BLK], F32, kind="ExternalInput").ap()
    ident = nc.dram_tensor("ident", [128, 128], BF16, kind="ExternalInput").ap()
    out = nc.dram_tensor("out", [D, T], BF16, kind="ExternalOutput").ap()

    with tile.TileContext(nc) as tc:
        with (
            tc.tile_pool(name="persist", bufs=1) as pp,
            tc.tile_pool(name="work", bufs=3) as wp,
            tc.tile_pool(name="psA", bufs=4, space="PSUM") as psA,
            tc.tile_pool(name="psO", bufs=2, space="PSUM") as psO,
            tc.tile_pool(name="psU", bufs=2, space="PSUM") as psU,
        ):
            # ---- persistent SBUF tiles -------------------------------------
            w1q_sb = pp.tile([128, ND * FH], BF16, name="w1q_sb", tag="w1q_sb")
            w1kv_sb = pp.tile([128, ND * 2 * FH], BF16, name="w1kv_sb", tag="w1kv_sb")
            b1q_sb = pp.tile([128, 2], F32, name="b1q_sb", tag="b1q_sb")
            bkv_sb = pp.tile([128, 512], F32, name="bkv_sb", tag="bkv_sb")
            m0_sb = pp.tile([128, BLK], F32, name="m0_sb", tag="m0_sb")
            m1_sb = pp.tile([128, BLK], F32, name="m1_sb", tag="m1_sb")
            id_sb = pp.tile([128, 128], BF16, name="id_sb", tag="id_sb")
            # per-head zero-padded Q^T: head h valid rows (h%2)*64..
            qTp = [pp.tile([128, T], BF16, name=f"qTp{h}", tag=f"qTp{h}") for h in range(4)]
            # packed K^T per head-pair pg: rows = 2 heads x 64 feats
            kT = [pp.tile([128, T], BF16, name=f"kT{g}", tag=f"kT{g}") for g in range(2)]
            # per 128-token chunk: [tok, K(256) | V(256)]
            kv_sb = [pp.tile([128, 512], BF16, name=f"kv{t}", tag=f"kv{t}") for t in range(NTC)]
            oT_sb = [pp.tile([128, T], BF16, name=f"oT{g}", tag=f"oT{g}") for g in range(2)]

            # zero the never-written halves of qTp once (DVE, idle at start)
            for h in range(4):
                par = h % 2
                nc.vector.memset(qTp[h][(1 - par) * 64:(2 - par) * 64, :], 0.0)

            with tc.tile_pool(name="xt", bufs=1) as xp:
                xt = [xp.tile([128, T], BF16, name=f"xt{i}", tag=f"xt{i}") for i in range(ND)]
                # small constants early, on the gpsimd queue
                nc.gpsimd.dma_start(out=m0_sb[:], in_=mask0[:])
                nc.gpsimd.dma_start(out=m1_sb[:], in_=mask1[:])
                nc.gpsimd.dma_start(out=id_sb[:], in_=ident[:])
                # Column-halved x^T stream: all chunks' first 1024 token-cols
                # land first so waves W1/W2 are runnable while the second
                # half streams.
                HT = T // 2
                for i in range(ND):
                    nc.sync.dma_start(out=xt[i][:, 0:HT], in_=xT[128 * i:128 * (i + 1), 0:HT])
                    nc.sync.dma_start(
                        out=w1q_sb[:, i * FH:(i + 1) * FH],
                        in_=w1q[128 * i:128 * (i + 1), :])
                    nc.gpsimd.dma_start(
                        out=w1kv_sb[:, i * 2 * FH:(i + 1) * 2 * FH],
                        in_=w1kv[128 * i:128 * (i + 1), :])
                    if i == 2:
                        nc.gpsimd.dma_start(out=b1q_sb[:], in_=b1q.rearrange("(c p) -> p c", p=128))
                        nc.gpsimd.dma_start(out=bkv_sb[:], in_=b1kv.unsqueeze(0).broadcast_to([128, 512]))
                for i in range(ND):
                    nc.sync.dma_start(out=xt[i][:, HT:T], in_=xT[128 * i:128 * (i + 1), HT:T])

                # ---- phase B: projections ----------------------------------
                # Waves of concurrent PSUM groups; within a wave the
                # contraction chunk d is the OUTER loop so the (in-order) PE
                # stream can run each chunk's matmuls as soon as that chunk
                # lands.
                _pools = [psA, psU, psO, psA, psU, psO, psA, psA]

                _ptag = {id(psA): "pa", id(psU): "pu", id(psO): "po"}

                def run_wave(groups, pools=None):
                    pl = pools if pools is not None else _pools
                    tiles = []
                    for gi, _ in enumerate(groups):
                        pool = pl[gi]
                        tiles.append(pool.tile([128, 512], F32, name=f"pw{gi}",
                                               tag=_ptag[id(pool)]))
                    for d in range(ND):
                        for gi, (lf, rf, _) in enumerate(groups):
                            nc.tensor.matmul(
                                tiles[gi][:], lf(d), rf(d),
                                start=(d == 0), stop=(d == ND - 1))
                    for gi, (_, _, cb) in enumerate(groups):
                        cb(tiles[gi])

                def q_group(ft, qt):
                    qsl = slice(qt * 512, (qt + 1) * 512)

                    def cb(pt):
                        # split the packed [128,512] result into the two
                        # per-head padded tiles; one half on ACT, one on DVE
                        h0, h1 = 2 * ft, 2 * ft + 1
                        nc.scalar.activation(
                            qTp[h0][0:64, qsl], pt[0:64, :],
                            AF.Identity, bias=b1q_sb[0:64, ft:ft + 1])
                        nc.vector.tensor_scalar_add(
                            qTp[h1][64:128, qsl], pt[64:128, :], b1q_sb[64:128, ft:ft + 1])
                    return (
                        lambda d: w1q_sb[:, d * FH + ft * 128: d * FH + (ft + 1) * 128],
                        lambda d: xt[d][:, qt * 512:(qt + 1) * 512],
                        cb)

                def kv_group(tcn):
                    def cb(pt):
                        nc.vector.tensor_tensor(
                            kv_sb[tcn][:], pt[:], bkv_sb[:], mybir.AluOpType.add)
                    return (
                        lambda d: xt[d][:, tcn * 128:(tcn + 1) * 128],
                        lambda d: w1kv_sb[:, d * 2 * FH:(d + 1) * 2 * FH],
                        cb)

                def transpose_quads(tc4s):
                    # one quad = 4 PE transposes of kv K chunks into one
                    # [128,512] PSUM tile; tc4s lists (pg, tc4) quads where
                    # tc4 covers token chunks 4*tc4 .. 4*tc4+3
                    pools = [psU, psU, psO, psO]
                    for qi, (pg, tc4) in enumerate(tc4s):
                        pool = pools[qi % 4]
                        # transpose output dtype must match its input (bf16)
                        qt_tile = pool.tile([128, 512], BF16, name="tq",
                                            tag=_ptag[id(pool)])
                        for k in range(4):
                            tcn = 4 * tc4 + k
                            nc.tensor.matmul(
                                qt_tile[:, k * 128:(k + 1) * 128],
                                kv_sb[tcn][:, pg * 128:(pg + 1) * 128],
                                id_sb[:], is_transpose=True,
                                start=True, stop=True, skip_group_check=(k > 0))
                        dst = kT[pg][:, tc4 * 512:(tc4 + 1) * 512]
                        if qi % 2 == 0:
                            nc.scalar.activation(dst, qt_tile[:], AF.Identity)
                        else:
                            nc.vector.tensor_copy(dst, qt_tile[:])

                # W1: Q(qt0,1) + KV(tcn0-3) — first-half tokens only
                run_wave([q_group(0, 0), q_group(0, 1), q_group(1, 0), q_group(1, 1),
                          kv_group(0), kv_group(1), kv_group(2), kv_group(3)])
                # W2: KV(tcn4-7) + 16 transposes (tcn0-7)
                run_wave([kv_group(4), kv_group(5), kv_group(6), kv_group(7)],
                         pools=[psA, psA, psA, psA])
                transpose_quads([(0, 0), (1, 0), (0, 1), (1, 1)])
                # W3: Q(qt2,3) + KV(tcn8-11) — second half
                run_wave([q_group(0, 2), q_group(0, 3), q_group(1, 2), q_group(1, 3),
                          kv_group(8), kv_group(9), kv_group(10), kv_group(11)])
                # W4: KV(tcn12-15) + 16 transposes (tcn8-15)
                run_wave([kv_group(12), kv_group(13), kv_group(14), kv_group(15)],
                         pools=[psA, psA, psA, psA])
                transpose_quads([(0, 2), (1, 2), (0, 3), (1, 3)])

            # ---- late pool: state tiles + W2 (reuses x^T space) ------------
            with tc.tile_pool(name="late", bufs=1) as lp:
                spad = [lp.tile([128, 128], BF16, name=f"spad{h}", tag=f"spad{h}") for h in range(4)]
                w2_sb = lp.tile([128, 2 * D], BF16, name="w2_sb", tag="w2_sb")
                # manually-rotated a1 ring: the left half of each slot is the
                # always-zero region of the chunk-1 scores; memset it once and
                # let the per-block mask multiply touch only the tril half.
                a1ring = [lp.tile([128, 2 * BLK], BF16, name=f"a1r{i}", tag=f"a1r{i}")
                          for i in range(4)]
                for i in range(4):
                    for par in range(2):
                        nc.vector.memset(a1ring[i][:, par * BLK: par * BLK + 128], 0.0)
                for h in range(4):
                    nc.vector.memset(spad[h][:], 0.0)
                nc.sync.dma_start(
                    out=w2_sb.rearrange("p (c f) -> p c f", c=2),
                    in_=w2.rearrange("(c p) f -> p c f", p=128))

                # ---- phase C: chunked causal attention + interleaved D -----
                # Two-stage software pipeline: block m's scores are emitted
                # before block m-1's O-accumulation chains, so the in-order PE
                # stream always has independent matmuls while the DVE applies
                # causal masks for the previous block.
                ablk = {}

                def scores_block(m):
                    qsl = slice(m * BLK, (m + 1) * BLK)
                    for pg in range(2):
                        a0 = lp.tile([128, 2 * BLK], BF16, name="a0", tag="a0", bufs=4)
                        a1 = a1ring[(2 * m + pg) % 4]
                        ablk[(m, pg)] = (a0, a1)
                        for par in range(2):
                            h = 2 * pg + par
                            asl = slice(par * BLK, (par + 1) * BLK)
                            pA = psA.tile([128, 2 * BLK], F32, name="pA", tag="pa")
                            nc.tensor.matmul(
                                pA[:, 0:BLK], kT[pg][:, (2 * m) * 128:(2 * m + 1) * 128],
                                qTp[h][:, qsl], start=True, stop=True)
                            nc.tensor.matmul(
                                pA[:, BLK:2 * BLK], kT[pg][:, (2 * m + 1) * 128:(2 * m + 2) * 128],
                                qTp[h][:, qsl], start=True, stop=True, skip_group_check=True)
                            nc.vector.tensor_tensor(a0[:, asl], pA[:, 0:BLK], m0_sb[:], mybir.AluOpType.mult)
                            # only the tril half: the left 128 cols stay zero
                            nc.vector.tensor_tensor(
                                a1[:, par * BLK + 128: (par + 1) * BLK],
                                pA[:, BLK + 128:2 * BLK], m1_sb[:, 128:BLK],
                                mybir.AluOpType.mult)

                def chains_block(m):
                    qsl = slice(m * BLK, (m + 1) * BLK)
                    for pg in range(2):
                        a0, a1 = ablk.pop((m, pg))
                        pO = psO.tile([128, 2 * BLK], F32, name="pO", tag="po")
                        nc.tensor.matmul(
                            pO[:], kv_sb[2 * m][:, FH + pg * 128: FH + (pg + 1) * 128],
                            a0[:], start=True, stop=False)
                        nc.tensor.matmul(
                            pO[:], kv_sb[2 * m + 1][:, FH + pg * 128: FH + (pg + 1) * 128],
                            a1[:], start=False, stop=(m == 0))
                        if m > 0:
                            nc.tensor.matmul(
                                pO[:, 0:BLK], spad[2 * pg][:], qTp[2 * pg][:, qsl],
                                start=False, stop=False)
                            nc.tensor.matmul(
                                pO[:, BLK:2 * BLK], spad[2 * pg + 1][:], qTp[2 * pg + 1][:, qsl],
                                start=False, stop=True)
                        for par in range(2):
                            hr = slice(par * 64, (par + 1) * 64)
                            src = pO[hr, par * BLK:par * BLK + BLK]
                            dst = oT_sb[pg][hr, qsl]
                            if par == 0:
                                nc.scalar.activation(dst, src, AF.Identity)
                            else:
                                nc.vector.tensor_copy(dst, src)

                    for pg in range(2):
                        pU = psU.tile([128, BLK], F32, name="pU", tag="pu")
                        nc.tensor.matmul(
                            pU[:], kv_sb[2 * m][:, pg * 128:(pg + 1) * 128],
                            kv_sb[2 * m][:, FH:2 * FH], start=True, stop=False)
                        nc.tensor.matmul(
                            pU[:], kv_sb[2 * m + 1][:, pg * 128:(pg + 1) * 128],
                            kv_sb[2 * m + 1][:, FH:2 * FH], start=False, stop=True)
                        for par in range(2):
                            h = 2 * pg + par
                            hr = slice(par * 64, (par + 1) * 64)
                            if MIXED_TT:
                                nc.vector.tensor_tensor(
                                    spad[h][hr, hr], pU[hr, h * 64:(h + 1) * 64],
                                    spad[h][hr, hr], mybir.AluOpType.add)
                            else:
                                tmp = wp.tile([64, 64], BF16, name="stmp", tag="stmp", bufs=2)
                                nc.vector.tensor_copy(tmp[:], pU[hr, h * 64:(h + 1) * 64])
                                nc.vector.tensor_tensor(
                                    spad[h][hr, hr], tmp[:],
                                    spad[h][hr, hr], mybir.AluOpType.add)

                def proj_tile(qt, half):
                    # two dout chunks (one dc pair) per copyback + DMA
                    tsl = slice(qt * 512, (qt + 1) * 512)
                    base = 0 if half == 0 else ND // 2
                    for pr in range(2):          # pairs within the half
                        dc0 = base + 2 * pr
                        pf = []
                        for j, dc in enumerate((dc0, dc0 + 1)):
                            pool = [psA, psU, psO][(dc + qt) % 3]
                            p = pool.tile([128, 512], F32, name="pf",
                                          tag=_ptag[id(pool)])
                            pf.append(p)
                            for g2 in range(2):
                                nc.tensor.matmul(
                                    p[:],
                                    w2_sb[:, g2 * D + dc * 128: g2 * D + (dc + 1) * 128],
                                    oT_sb[g2][:, tsl],
                                    start=(g2 == 0), stop=(g2 == 1))
                        fs = lp.tile([128, 2, 512], BF16, name="fs", tag="fs", bufs=3)
                        if pr == 0:
                            nc.vector.tensor_copy(fs[:, 0, :], pf[0][:])
                            nc.vector.tensor_copy(fs[:, 1, :], pf[1][:])
                        else:
                            nc.scalar.activation(fs[:, 0, :], pf[0][:], AF.Identity)
                            nc.scalar.activation(fs[:, 1, :], pf[1][:], AF.Identity)
                        dma_eng = nc.gpsimd if pr == 0 else nc.sync
                        dma_eng.dma_start(
                            out=out[dc0 * 128:(dc0 + 2) * 128, tsl].rearrange(
                                "(c p) t -> p c t", p=128),
                            in_=fs[:])

                def proj_tile256(tcn):
                    tsl = slice(tcn * 128, (tcn + 2) * 128)  # 256 tokens
                    for pr in range(4):
                        dc0 = 2 * pr
                        pf = []
                        for j, dc in enumerate((dc0, dc0 + 1)):
                            pool = [psA, psU, psO][dc % 3]
                            p = pool.tile([128, 512], F32, name="pf2",
                                          tag=_ptag[id(pool)])
                            pf.append(p)
                            for g2 in range(2):
                                nc.tensor.matmul(
                                    p[:, 0:BLK],
                                    w2_sb[:, g2 * D + dc * 128: g2 * D + (dc + 1) * 128],
                                    oT_sb[g2][:, tsl],
                                    start=(g2 == 0), stop=(g2 == 1))
                        fs = lp.tile([128, 2, BLK], BF16, name="fs2", tag="fs2", bufs=3)
                        if pr % 2 == 0:
                            nc.vector.tensor_copy(fs[:, 0, :], pf[0][:, 0:BLK])
                            nc.vector.tensor_copy(fs[:, 1, :], pf[1][:, 0:BLK])
                        else:
                            nc.scalar.activation(fs[:, 0, :], pf[0][:, 0:BLK], AF.Identity)
                            nc.scalar.activation(fs[:, 1, :], pf[1][:, 0:BLK], AF.Identity)
                        dma_eng = nc.gpsimd if pr % 2 == 0 else nc.sync
                        dma_eng.dma_start(
                            out=out[dc0 * 128:(dc0 + 2) * 128, tsl].rearrange(
                                "(c p) t -> p c t", p=128),
                            in_=fs[:])

                # proj tiles are emitted one block after the copybacks that
                # produce their oT inputs; the last two blocks go out at
                # 256-token granularity to shrink the serial tail.
                dplan = {2: (0, 0), 3: (0, 1), 4: (1, 0), 5: (1, 1), 6: (2, 0), 7: (2, 1)}
                scores_block(0)
                for m in range(1, NBLK):
                    scores_block(m)
                    chains_block(m - 1)
                    if m in dplan:
                        proj_tile(*dplan[m])
                    if m == NBLK - 1:
                        proj_tile256(12)   # block 6 (tokens 1536:1792)
                chains_block(NBLK - 1)
                proj_tile256(14)           # block 7 (tokens 1792:2048)

    nc.compile()
    return nc


_NC = None


def _get_nc():
    global _NC
    if _NC is None:
        _NC = _build()
    return _NC


def make_core_inputs(x, W1, b1, W2, b2):
    """Shard full inputs into the 8 per-core input dicts."""
    x = np.asarray(x, dtype=np.float32)
    W1 = np.asarray(W1, dtype=np.float32)
    b1 = np.asarray(b1, dtype=np.float32)
    W2 = np.asarray(W2, dtype=np.float32)

    p = np.arange(128)[:, None]
    f = np.arange(BLK)[None, :]
    mask0 = (f >= p).astype(np.float32)
    mask1 = (f >= p + 128).astype(np.float32)
    ident = np.eye(128, dtype=np.float32).astype(BF)

    in_maps = []
    for c in range(8):
        b = c // 4
        g = c % 4
        ksl = slice(g * FH, (g + 1) * FH)
        qsl = slice(D + g * FH, D + (g + 1) * FH)
        vsl = slice(2 * D + g * FH, 2 * D + (g + 1) * FH)
        in_maps.append({
            "xT": np.ascontiguousarray(x[b].T).astype(BF),
            "w1q": np.ascontiguousarray(W1[:, qsl]).astype(BF),
            "w1kv": np.ascontiguousarray(
                np.concatenate([W1[:, ksl], W1[:, vsl]], axis=1)).astype(BF),
            "b1q": np.ascontiguousarray(b1[qsl]),
            "b1kv": np.ascontiguousarray(np.concatenate([b1[ksl], b1[vsl]])),
            "w2": np.ascontiguousarray(W2[ksl, :]).astype(BF),
            "mask0": mask0,
            "mask1": mask1,
            "ident": ident,
        })
    return in_maps


def kernel(x, W1, b1, W2, b2):
    nc = _get_nc()
    in_maps = make_core_inputs(x, W1, b1, W2, b2)
    kwargs = {}
    if TRACE:
        kwargs = {"trace": True, "tmpdir": TRACE_DIR}
    res = run_bass_kernel_spmd(nc, in_maps, list(range(8)), **kwargs)
    LAST_RESULTS[0] = res
    b2 = np.asarray(b2, dtype=np.float32)
    out = np.zeros((B, T, D), np.float32)
    for c in range(8):
        out[c // 4] += res.results[c]["out"].astype(np.float32).T
    out += b2[None, None, :]
    return out


# revision 4
# speedup vs baseline: 1.2992x; 1.0379x over previous
"""Trainium2 Bass kernel for causal softmax-free multi-head attention (retention).

Reference computation (per batch b):
    kqv = x @ W1 + b1 ; k, q, v = split(kqv, 3)   [split order k, q, v]
    per head h (dh = 64):  attn = tril(q_h @ k_h^T) ; o_h = attn @ v_h
    out = concat_h(o_h) @ W2 + b2

Sharding: 8 cores = 2 batches x 4 head-groups (4 heads each). Each core
computes its batch's projections restricted to its heads' weight columns,
the attention for its 4 heads, and a partial output projection
(out_local @ W2[rows of its heads]). Host sums the 4 partials per batch.

v2 vs v1:
  - all matmul operands are bf16 (PSUM accumulation stays f32): halves DMA
    traffic in (x/W1/W2 stream as bf16) and out (bf16 partials), and halves
    DVE element time. Max rel err vs f32 reference ~6e-3 (gate 2e-2).
  - K is projected ONCE ([tok, feat] layout inside the KV wave); the
    [feat, tok] layout needed by the score matmuls comes from PE transposes
    (32 x [128,128] via identity), replacing the v1 duplicate K projection
    (32768 PE rows -> 4096).
  - scores select heads by zero-padding Q (qTp[h], memset halves) instead
    of zero-padding K, so the transposed K stays packed.
  - zero fills via engine memset, not DMA (kills the zer input stream).
  - output projection spread earlier across phase C; the last two blocks
    are emitted at 256-token granularity so the tail after the final
    chains is only 2 small proj groups; out-DMAs pair two dout chunks
    per transfer ([128, 2, 256/512] APs) to halve trigger count.

Algorithm: chunked linear attention. tril(QK^T)V is computed per 256-token
block as  O = Q @ S + tril_block(Q K_blk^T) V_blk, with the running state
S = K^T V accumulated over previous blocks ([64,64] per head).

Hardware constraints honored:
  - matmul tiles: K=128 contraction, M=128 stationary, N>=256 moving
    (N=128 only for the PE transposes, which are exact data movement).
  - DMA cannot touch PSUM: every matmul result is copied out via DVE/ACT.
"""

import numpy as np
import ml_dtypes

import concourse.bacc as bacc
import concourse.mybir as mybir
import concourse.tile as tile
from concourse.bass_utils import run_bass_kernel_spmd

F32 = mybir.dt.float32
BF16 = mybir.dt.bfloat16
AF = mybir.ActivationFunctionType
BF = ml_dtypes.bfloat16

B, T, D = 2, 2048, 1024
H, DH = 16, 64
HPC = 4           # heads per core
FH = HPC * DH     # 256 features per core per tensor
BLK = 256         # state-update block (2 x 128-token chunks)
NBLK = T // BLK   # 8
NTC = T // 128    # 16 token chunks
ND = D // 128     # 8 contraction chunks

# set False if mixed-dtype (f32 PSUM + bf16 SBUF) tensor_tensor misbehaves
MIXED_TT = True

TRACE = False
TRACE_DIR = None
LAST_RESULTS = [None]


def _build():
    nc = bacc.Bacc("TRN2", target_bir_lowering=False, debug=False, num_devices=8)

    xT = nc.dram_tensor("xT", [D, T], BF16, kind="ExternalInput").ap()
    w1q = nc.dram_tensor("w1q", [D, FH], BF16, kind="ExternalInput").ap()
    w1kv = nc.dram_tensor("w1kv", [D, 2 * FH], BF16, kind="ExternalInput").ap()
    b1q = nc.dram_tensor("b1q", [FH], F32, kind="ExternalInput").ap()
    b1kv = nc.dram_tensor("b1kv", [2 * FH], F32, kind="ExternalInput").ap()
    w2 = nc.dram_tensor("w2", [FH, D], BF16, kind="ExternalInput").ap()
    mask0 = nc.dram_tensor("mask0", [128, BLK], F32, kind="ExternalInput").ap()
    mask1 = nc.dram_tensor("mask1", [128, BLK], F32, kind="ExternalInput").ap()
    ident = nc.dram_tensor("ident", [128, 128], BF16, kind="ExternalInput").ap()
    out = nc.dram_tensor("out", [D, T], BF16, kind="ExternalOutput").ap()

    with tile.TileContext(nc) as tc:
        with (
            tc.tile_pool(name="persist", bufs=1) as pp,
            tc.tile_pool(name="work", bufs=3) as wp,
            tc.tile_pool(name="psA", bufs=4, space="PSUM") as psA,
            tc.tile_pool(name="psO", bufs=2, space="PSUM") as psO,
            tc.tile_pool(name="psU", bufs=2, space="PSUM") as psU,
        ):
            # ---- persistent SBUF tiles -------------------------------------
            w1q_sb = pp.tile([128, ND * FH], BF16, name="w1q_sb", tag="w1q_sb")
            w1kv_sb = pp.tile([128, ND * 2 * FH], BF16, name="w1kv_sb", tag="w1kv_sb")
            b1q_sb = pp.tile([128, 2], F32, name="b1q_sb", tag="b1q_sb")
            bkv_sb = pp.tile([128, 512], F32, name="bkv_sb", tag="bkv_sb")
            m0_sb = pp.tile([128, BLK], F32, name="m0_sb", tag="m0_sb")
            m1_sb = pp.tile([128, BLK], F32, name="m1_sb", tag="m1_sb")
            id_sb = pp.tile([128, 128], BF16, name="id_sb", tag="id_sb")
            # per-head zero-padded Q^T: head h valid rows (h%2)*64..
            qTp = [pp.tile([128, T], BF16, name=f"qTp{h}", tag=f"qTp{h}") for h in range(4)]
            # packed K^T per head-pair pg: rows = 2 heads x 64 feats
            kT = [pp.tile([128, T], BF16, name=f"kT{g}", tag=f"kT{g}") for g in range(2)]
            # per 128-token chunk: [tok, K(256) | V(256)]
            kv_sb = [pp.tile([128, 512], BF16, name=f"kv{t}", tag=f"kv{t}") for t in range(NTC)]
            oT_sb = [pp.tile([128, T], BF16, name=f"oT{g}", tag=f"oT{g}") for g in range(2)]

            with tc.tile_pool(name="xt", bufs=1) as xp:
                xt = [xp.tile([128, T], BF16, name=f"xt{i}", tag=f"xt{i}") for i in range(ND)]
                # A single DMA transfer runs on ONE of the 16 DMA engines at
                # ~22 GB/s, so per-transfer latency (not queue trigger cost)
                # gates the head.  Split the first chunks into small pieces
                # across FOUR issuing queues so W1's first contraction chunk
                # is resident ~3.5us in instead of ~11us.
                HT = T // 2

                def xrow(i):
                    return xT[128 * i:128 * (i + 1), :]

                # zero the never-written halves of qTp (vector issues no DMAs,
                # so these run immediately)
                for h in range(4):
                    par = h % 2
                    nc.vector.memset(qTp[h][(1 - par) * 64:(2 - par) * 64, :], 0.0)
                # first chunk pieces first so W1's d=0 matmuls can start ASAP
                nc.sync.dma_start(out=w1q_sb[:, 0:FH], in_=w1q[0:128, :])
                nc.scalar.dma_start(out=xt[0][:, 0:512], in_=xrow(0)[:, 0:512])
                nc.gpsimd.dma_start(out=w1kv_sb[:, 0:2 * FH], in_=w1kv[0:128, :])
                nc.scalar.dma_start(out=xt[0][:, 512:HT], in_=xrow(0)[:, 512:HT])
                nc.scalar.dma_start(out=b1q_sb[:], in_=b1q.rearrange("(c p) -> p c", p=128))
                # chunks 1-7: x first-half in 2 pieces (sync+scalar), w1q on
                # sync, w1kv on gpsimd
                for i in range(1, ND):
                    nc.sync.dma_start(out=xt[i][:, 0:512], in_=xrow(i)[:, 0:512])
                    nc.scalar.dma_start(out=xt[i][:, 512:HT], in_=xrow(i)[:, 512:HT])
                    nc.sync.dma_start(
                        out=w1q_sb[:, i * FH:(i + 1) * FH],
                        in_=w1q[128 * i:128 * (i + 1), :])
                    nc.gpsimd.dma_start(
                        out=w1kv_sb[:, i * 2 * FH:(i + 1) * 2 * FH],
                        in_=w1kv[128 * i:128 * (i + 1), :])
                # constants needed by copybacks / phase C
                nc.gpsimd.dma_start(out=bkv_sb[:], in_=b1kv.unsqueeze(0).broadcast_to([128, 512]))
                nc.gpsimd.dma_start(out=m0_sb[:], in_=mask0[:])
                nc.gpsimd.dma_start(out=m1_sb[:], in_=mask1[:])
                nc.gpsimd.dma_start(out=id_sb[:], in_=ident[:])
                # second halves, two pieces each, on sync/gpsimd (the scalar
                # queue is needed for W1 copybacks by now)
                for i in range(ND):
                    nc.sync.dma_start(out=xt[i][:, HT:HT + 512], in_=xrow(i)[:, HT:HT + 512])
                    nc.gpsimd.dma_start(out=xt[i][:, HT + 512:T], in_=xrow(i)[:, HT + 512:T])

                # ---- phase B: projections ----------------------------------
                # Waves of concurrent PSUM groups; within a wave the
                # contraction chunk d is the OUTER loop so the (in-order) PE
                # stream can run each chunk's matmuls as soon as that chunk
                # lands.
                _pools = [psA, psU, psO, psA, psU, psO, psA, psA]

                _ptag = {id(psA): "pa", id(psU): "pu", id(psO): "po"}

                def run_wave(groups, pools=None):
                    pl = pools if pools is not None else _pools
                    tiles = []
                    for gi, _ in enumerate(groups):
                        pool = pl[gi]
                        tiles.append(pool.tile([128, 512], F32, name=f"pw{gi}",
                                               tag=_ptag[id(pool)]))
                    for d in range(ND):
                        for gi, (lf, rf, _) in enumerate(groups):
                            nc.tensor.matmul(
                                tiles[gi][:], lf(d), rf(d),
                                start=(d == 0), stop=(d == ND - 1))
                    for gi, (_, _, cb) in enumerate(groups):
                        cb(tiles[gi])

                def q_group(ft, qt):
                    qsl = slice(qt * 512, (qt + 1) * 512)

                    def cb(pt):
                        # split the packed [128,512] result into the two
                        # per-head padded tiles (both on ACT; DVE is loaded)
                        h0, h1 = 2 * ft, 2 * ft + 1
                        nc.scalar.activation(
                            qTp[h0][0:64, qsl], pt[0:64, :],
                            AF.Identity, bias=b1q_sb[0:64, ft:ft + 1])
                        nc.scalar.activation(
                            qTp[h1][64:128, qsl], pt[64:128, :],
                            AF.Identity, bias=b1q_sb[64:128, ft:ft + 1])
                    return (
                        lambda d: w1q_sb[:, d * FH + ft * 128: d * FH + (ft + 1) * 128],
                        lambda d: xt[d][:, qt * 512:(qt + 1) * 512],
                        cb)

                def kv_group(tcn):
                    def cb(pt):
                        nc.vector.tensor_tensor(
                            kv_sb[tcn][:], pt[:], bkv_sb[:], mybir.AluOpType.add)
                    return (
                        lambda d: xt[d][:, tcn * 128:(tcn + 1) * 128],
                        lambda d: w1kv_sb[:, d * 2 * FH:(d + 1) * 2 * FH],
                        cb)

                def transpose_quads(tc4s):
                    # one quad = 4 PE transposes of kv K chunks into one
                    # [128,512] PSUM tile; tc4s lists (pg, tc4) quads where
                    # tc4 covers token chunks 4*tc4 .. 4*tc4+3
                    pools = [psU, psU, psO, psO]
                    for qi, (pg, tc4) in enumerate(tc4s):
                        pool = pools[qi % 4]
                        # transpose output dtype must match its input (bf16)
                        qt_tile = pool.tile([128, 512], BF16, name="tq",
                                            tag=_ptag[id(pool)])
                        for k in range(4):
                            tcn = 4 * tc4 + k
                            nc.tensor.matmul(
                                qt_tile[:, k * 128:(k + 1) * 128],
                                kv_sb[tcn][:, pg * 128:(pg + 1) * 128],
                                id_sb[:], is_transpose=True,
                                start=True, stop=True, skip_group_check=(k > 0))
                        dst = kT[pg][:, tc4 * 512:(tc4 + 1) * 512]
                        if qi % 2 == 0:
                            nc.scalar.activation(dst, qt_tile[:], AF.Identity)
                        else:
                            nc.vector.tensor_copy(dst, qt_tile[:])

                # W1: Q(qt0,1) + KV(tcn0-3) — first-half tokens only
                run_wave([q_group(0, 0), q_group(0, 1), q_group(1, 0), q_group(1, 1),
                          kv_group(0), kv_group(1), kv_group(2), kv_group(3)])
                # W2: KV(tcn4-7) + 16 transposes (tcn0-7)
                run_wave([kv_group(4), kv_group(5), kv_group(6), kv_group(7)],
                         pools=[psA, psA, psA, psA])
                transpose_quads([(0, 0), (1, 0), (0, 1), (1, 1)])
                # W3: Q(qt2,3) + KV(tcn8-11) — second half
                run_wave([q_group(0, 2), q_group(0, 3), q_group(1, 2), q_group(1, 3),
                          kv_group(8), kv_group(9), kv_group(10), kv_group(11)])
                # W4: KV(tcn12-15) + 16 transposes (tcn8-15)
                run_wave([kv_group(12), kv_group(13), kv_group(14), kv_group(15)],
                         pools=[psA, psA, psA, psA])
                transpose_quads([(0, 2), (1, 2), (0, 3), (1, 3)])

            # ---- late pool: state tiles + W2 (reuses x^T space) ------------
            with tc.tile_pool(name="late", bufs=1) as lp:
                # one state tile per head pair: diagonal 64x64 blocks hold the
                # two heads' running K^T V; the off-diagonal blocks accumulate
                # cross-head garbage that the Q@S matmuls never touch (M=64
                # column slices + zero-padded qTp rows)
                spad = [lp.tile([128, 128], BF16, name=f"spad{g}", tag=f"spad{g}") for g in range(2)]
                w2_sb = lp.tile([128, 2 * D], BF16, name="w2_sb", tag="w2_sb")
                # manually-rotated a1 ring: the left half of each slot is the
                # always-zero region of the chunk-1 scores; memset it once and
                # let the per-block mask multiply touch only the tril half.
                a1ring = [lp.tile([128, 2 * BLK], BF16, name=f"a1r{i}", tag=f"a1r{i}")
                          for i in range(4)]
                for i in range(4):
                    for par in range(2):
                        nc.gpsimd.memset(a1ring[i][:, par * BLK: par * BLK + 128], 0.0)
                for g in range(2):
                    nc.gpsimd.memset(spad[g][:], 0.0)
                nc.sync.dma_start(
                    out=w2_sb.rearrange("p (c f) -> p c f", c=2),
                    in_=w2.rearrange("(c p) f -> p c f", p=128))

                # ---- phase C: chunked causal attention + interleaved D -----
                # Two-stage software pipeline: block m's scores are emitted
                # before block m-1's O-accumulation chains, so the in-order PE
                # stream always has independent matmuls while the DVE applies
                # causal masks for the previous block.
                ablk = {}

                def scores_block(m):
                    qsl = slice(m * BLK, (m + 1) * BLK)
                    qsl2 = slice(m * BLK + 128, (m + 1) * BLK)
                    for pg in range(2):
                        a0 = lp.tile([128, 2 * BLK], BF16, name="a0", tag="a0", bufs=4)
                        a1 = a1ring[(2 * m + pg) % 4]
                        ablk[(m, pg)] = (a0, a1)
                        for par in range(2):
                            h = 2 * pg + par
                            asl = slice(par * BLK, (par + 1) * BLK)
                            pA = psA.tile([128, 2 * BLK], F32, name="pA", tag="pa")
                            nc.tensor.matmul(
                                pA[:, 0:BLK], kT[pg][:, (2 * m) * 128:(2 * m + 1) * 128],
                                qTp[h][:, qsl], start=True, stop=True)
                            # chunk 2m+1 only sees the last 128 qtoks (N=128)
                            nc.tensor.matmul(
                                pA[:, BLK:BLK + 128], kT[pg][:, (2 * m + 1) * 128:(2 * m + 2) * 128],
                                qTp[h][:, qsl2], start=True, stop=True, skip_group_check=True)
                            nc.vector.tensor_tensor(a0[:, asl], pA[:, 0:BLK], m0_sb[:], mybir.AluOpType.mult)
                            # only the tril half: the left 128 cols stay zero
                            # (GPSIMD cannot read PSUM, so this stays on DVE)
                            nc.vector.tensor_tensor(
                                a1[:, par * BLK + 128: (par + 1) * BLK],
                                pA[:, BLK:BLK + 128], m1_sb[:, 128:BLK],
                                mybir.AluOpType.mult)

                def chains_block(m):
                    qsl = slice(m * BLK, (m + 1) * BLK)
                    for pg in range(2):
                        a0, a1 = ablk.pop((m, pg))
                        # AV uses per-head M=64 stationary V slices so the
                        # accumulated pO rows are clean/packed: one copyback
                        # per head pair instead of two strided halves; the
                        # Q@S terms use M=64 column slices of the pair state.
                        pO = psO.tile([128, BLK], F32, name="pO", tag="po")
                        # all skip_group_check: the interp's zero-region
                        # bookkeeping cannot represent partition-offset PSUM
                        # writes (real ordering is enforced by tile deps; the
                        # hardware start bits are still per-instruction)
                        for par in range(2):
                            vsl = slice(FH + pg * 128 + par * 64, FH + pg * 128 + (par + 1) * 64)
                            hr = slice(par * 64, (par + 1) * 64)
                            asl = slice(par * BLK, (par + 1) * BLK)
                            nc.tensor.matmul(
                                pO[hr, :], kv_sb[2 * m][:, vsl], a0[:, asl],
                                start=True, stop=False, skip_group_check=True)
                            nc.tensor.matmul(
                                pO[hr, :], kv_sb[2 * m + 1][:, vsl], a1[:, asl],
                                start=False, stop=False, skip_group_check=True)
                        nc.tensor.matmul(
                            pO[0:64, :], spad[pg][:, 0:64], qTp[2 * pg][:, qsl],
                            start=False, stop=True, skip_group_check=True)
                        nc.tensor.matmul(
                            pO[64:128, :], spad[pg][:, 64:128], qTp[2 * pg + 1][:, qsl],
                            start=False, stop=True, skip_group_check=True)
                        nc.scalar.activation(oT_sb[pg][:, qsl], pO[:], AF.Identity)

                    for pg in range(2):
                        # S update restricted to this head pair: out [128,128]
                        # whose two diagonal 64x64 blocks are the per-head
                        # K^T V increments (off-diagonal garbage is harmless)
                        vpg = slice(FH + pg * 128, FH + (pg + 1) * 128)
                        pU = psU.tile([128, 128], F32, name="pU", tag="pu")
                        nc.tensor.matmul(
                            pU[:], kv_sb[2 * m][:, pg * 128:(pg + 1) * 128],
                            kv_sb[2 * m][:, vpg], start=True, stop=False)
                        nc.tensor.matmul(
                            pU[:], kv_sb[2 * m + 1][:, pg * 128:(pg + 1) * 128],
                            kv_sb[2 * m + 1][:, vpg], start=False, stop=True)
                        nc.vector.tensor_tensor(
                            spad[pg][:], pU[:], spad[pg][:], mybir.AluOpType.add)

                def proj_tile(qt, half):
                    # two dout chunks (one dc pair) per copyback + DMA
                    tsl = slice(qt * 512, (qt + 1) * 512)
                    base = 0 if half == 0 else ND // 2
                    for pr in range(2):          # pairs within the half
                        dc0 = base + 2 * pr
                        pf = []
                        for j, dc in enumerate((dc0, dc0 + 1)):
                            pool = [psA, psU, psO][(dc + qt) % 3]
                            p = pool.tile([128, 512], F32, name="pf",
                                          tag=_ptag[id(pool)])
                            pf.append(p)
                            for g2 in range(2):
                                nc.tensor.matmul(
                                    p[:],
                                    w2_sb[:, g2 * D + dc * 128: g2 * D + (dc + 1) * 128],
                                    oT_sb[g2][:, tsl],
                                    start=(g2 == 0), stop=(g2 == 1))
                        fs = lp.tile([128, 2, 512], BF16, name="fs", tag="fs", bufs=3)
                        if pr == 0:
                            nc.vector.tensor_copy(fs[:, 0, :], pf[0][:])
                            nc.vector.tensor_copy(fs[:, 1, :], pf[1][:])
                        else:
                            nc.scalar.activation(fs[:, 0, :], pf[0][:], AF.Identity)
                            nc.scalar.activation(fs[:, 1, :], pf[1][:], AF.Identity)
                        dma_eng = nc.gpsimd if pr == 0 else nc.sync
                        dma_eng.dma_start(
                            out=out[dc0 * 128:(dc0 + 2) * 128, tsl].rearrange(
                                "(c p) t -> p c t", p=128),
                            in_=fs[:])

                def proj_tile256(tcn):
                    tsl = slice(tcn * 128, (tcn + 2) * 128)  # 256 tokens
                    for pr in range(4):
                        dc0 = 2 * pr
                        pf = []
                        for j, dc in enumerate((dc0, dc0 + 1)):
                            pool = [psA, psU, psO][dc % 3]
                            p = pool.tile([128, 512], F32, name="pf2",
                                          tag=_ptag[id(pool)])
                            pf.append(p)
                            for g2 in range(2):
                                nc.tensor.matmul(
                                    p[:, 0:BLK],
                                    w2_sb[:, g2 * D + dc * 128: g2 * D + (dc + 1) * 128],
                                    oT_sb[g2][:, tsl],
                                    start=(g2 == 0), stop=(g2 == 1))
                        fs = lp.tile([128, 2, BLK], BF16, name="fs2", tag="fs2", bufs=3)
                        if pr % 2 == 0:
                            nc.vector.tensor_copy(fs[:, 0, :], pf[0][:, 0:BLK])
                            nc.vector.tensor_copy(fs[:, 1, :], pf[1][:, 0:BLK])
                        else:
                            nc.scalar.activation(fs[:, 0, :], pf[0][:, 0:BLK], AF.Identity)
                            nc.scalar.activation(fs[:, 1, :], pf[1][:, 0:BLK], AF.Identity)
                        dma_eng = nc.gpsimd if pr % 2 == 0 else nc.sync
                        dma_eng.dma_start(
                            out=out[dc0 * 128:(dc0 + 2) * 128, tsl].rearrange(
                                "(c p) t -> p c t", p=128),
                            in_=fs[:])

                # proj tiles are emitted one block after the copybacks that
                # produce their oT inputs; the last two blocks go out at
                # 256-token granularity to shrink the serial tail.
                dplan = {2: (0, 0), 3: (0, 1), 4: (1, 0), 5: (1, 1), 6: (2, 0), 7: (2, 1)}
                scores_block(0)
                for m in range(1, NBLK):
                    scores_block(m)
                    chains_block(m - 1)
                    if m in dplan:
                        proj_tile(*dplan[m])
                    if m == NBLK - 1:
                        proj_tile256(12)   # block 6 (tokens 1536:1792)
                chains_block(NBLK - 1)
                proj_tile256(14)           # block 7 (tokens 1792:2048)

    nc.compile()
    return nc


_NC = None


def _get_nc():
    global _NC
    if _NC is None:
        _NC = _build()
    return _NC


def make_core_inputs(x, W1, b1, W2, b2):
    """Shard full inputs into the 8 per-core input dicts."""
    x = np.asarray(x, dtype=np.float32)
    W1 = np.asarray(W1, dtype=np.float32)
    b1 = np.asarray(b1, dtype=np.float32)
    W2 = np.asarray(W2, dtype=np.float32)

    p = np.arange(128)[:, None]
    f = np.arange(BLK)[None, :]
    mask0 = (f >= p).astype(np.float32)
    mask1 = (f >= p + 128).astype(np.float32)
    ident = np.eye(128, dtype=np.float32).astype(BF)

    in_maps = []
    for c in range(8):
        b = c // 4
        g = c % 4
        ksl = slice(g * FH, (g + 1) * FH)
        qsl = slice(D + g * FH, D + (g + 1) * FH)
        vsl = slice(2 * D + g * FH, 2 * D + (g + 1) * FH)
        in_maps.append({
            "xT": np.ascontiguousarray(x[b].T).astype(BF),
            "w1q": np.ascontiguousarray(W1[:, qsl]).astype(BF),
            "w1kv": np.ascontiguousarray(
                np.concatenate([W1[:, ksl], W1[:, vsl]], axis=1)).astype(BF),
            "b1q": np.ascontiguousarray(b1[qsl]),
            "b1kv": np.ascontiguousarray(np.concatenate([b1[ksl], b1[vsl]])),
            "w2": np.ascontiguousarray(W2[ksl, :]).astype(BF),
            "mask0": mask0,
            "mask1": mask1,
            "ident": ident,
        })
    return in_maps


def kernel(x, W1, b1, W2, b2):
    nc = _get_nc()
    in_maps = make_core_inputs(x, W1, b1, W2, b2)
    kwargs = {}
    if TRACE:
        kwargs = {"trace": True, "tmpdir": TRACE_DIR}
    res = run_bass_kernel_spmd(nc, in_maps, list(range(8)), **kwargs)
    LAST_RESULTS[0] = res
    b2 = np.asarray(b2, dtype=np.float32)
    out = np.zeros((B, T, D), np.float32)
    for c in range(8):
        out[c // 4] += res.results[c]["out"].astype(np.float32).T
    out += b2[None, None, :]
    return out


# revision 5
# speedup vs baseline: 1.5423x; 1.1871x over previous
"""Trainium2 Bass kernel for causal softmax-free multi-head attention (retention).

Reference computation (per batch b):
    kqv = x @ W1 + b1 ; k, q, v = split(kqv, 3)   [split order k, q, v]
    per head h (dh = 64):  attn = tril(q_h @ k_h^T) ; o_h = attn @ v_h
    out = concat_h(o_h) @ W2 + b2

Sharding: 8 cores = 2 batches x 4 head-groups (4 heads each). Each core
computes its batch's projections restricted to its heads' weight columns,
the attention for its 4 heads, and a partial output projection
(out_local @ W2[rows of its heads]). Host sums the 4 partials per batch.

v2 vs v1:
  - all matmul operands are bf16 (PSUM accumulation stays f32): halves DMA
    traffic in (x/W1/W2 stream as bf16) and out (bf16 partials), and halves
    DVE element time. Max rel err vs f32 reference ~6e-3 (gate 2e-2).
  - K is projected ONCE ([tok, feat] layout inside the KV wave); the
    [feat, tok] layout needed by the score matmuls comes from PE transposes
    (32 x [128,128] via identity), replacing the v1 duplicate K projection
    (32768 PE rows -> 4096).
  - scores select heads by zero-padding Q (qTp[h], memset halves) instead
    of zero-padding K, so the transposed K stays packed.
  - zero fills via engine memset, not DMA (kills the zer input stream).
  - output projection spread earlier across phase C; the last two blocks
    are emitted at 256-token granularity so the tail after the final
    chains is only 2 small proj groups; out-DMAs pair two dout chunks
    per transfer ([128, 2, 256/512] APs) to halve trigger count.

Algorithm: chunked linear attention. tril(QK^T)V is computed per 256-token
block as  O = Q @ S + tril_block(Q K_blk^T) V_blk, with the running state
S = K^T V accumulated over previous blocks ([64,64] per head).

Hardware constraints honored:
  - matmul tiles: K=128 contraction, M=128 stationary, N>=256 moving
    (N=128 only for the PE transposes, which are exact data movement).
  - DMA cannot touch PSUM: every matmul result is copied out via DVE/ACT.
"""

import numpy as np
import ml_dtypes

import concourse.bacc as bacc
import concourse.mybir as mybir
import concourse.tile as tile
from concourse.bass_utils import run_bass_kernel_spmd

F32 = mybir.dt.float32
BF16 = mybir.dt.bfloat16
AF = mybir.ActivationFunctionType
BF = ml_dtypes.bfloat16

B, T, D = 2, 2048, 1024
H, DH = 16, 64
HPC = 4           # heads per core
FH = HPC * DH     # 256 features per core per tensor
BLK = 256         # state-update block (2 x 128-token chunks)
NBLK = T // BLK   # 8
NTC = T // 128    # 16 token chunks
ND = D // 128     # 8 contraction chunks

# set False if mixed-dtype (f32 PSUM + bf16 SBUF) tensor_tensor misbehaves
MIXED_TT = True

TRACE = False
TRACE_DIR = None
LAST_RESULTS = [None]


def _build():
    nc = bacc.Bacc("TRN2", target_bir_lowering=False, debug=False, num_devices=8)

    xT = nc.dram_tensor("xT", [D, T], BF16, kind="ExternalInput").ap()
    w1q = nc.dram_tensor("w1q", [D, FH], BF16, kind="ExternalInput").ap()
    w1kv = nc.dram_tensor("w1kv", [D, 2 * FH], BF16, kind="ExternalInput").ap()
    b1q = nc.dram_tensor("b1q", [FH], F32, kind="ExternalInput").ap()
    b1kv = nc.dram_tensor("b1kv", [2 * FH], F32, kind="ExternalInput").ap()
    w2 = nc.dram_tensor("w2", [FH, D], BF16, kind="ExternalInput").ap()
    mask0 = nc.dram_tensor("mask0", [128, BLK], F32, kind="ExternalInput").ap()
    mask1 = nc.dram_tensor("mask1", [128, BLK], F32, kind="ExternalInput").ap()
    ident = nc.dram_tensor("ident", [128, 128], BF16, kind="ExternalInput").ap()
    out = nc.dram_tensor("out", [D, T], BF16, kind="ExternalOutput").ap()

    with tile.TileContext(nc) as tc:
        with (
            tc.tile_pool(name="persist", bufs=1) as pp,
            tc.tile_pool(name="work", bufs=3) as wp,
            tc.tile_pool(name="psA", bufs=4, space="PSUM") as psA,
            tc.tile_pool(name="psO", bufs=2, space="PSUM") as psO,
            tc.tile_pool(name="psU", bufs=2, space="PSUM") as psU,
        ):
            # ---- persistent SBUF tiles -------------------------------------
            w1q_sb = pp.tile([128, ND * FH], BF16, name="w1q_sb", tag="w1q_sb")
            w1kv_sb = pp.tile([128, ND * 2 * FH], BF16, name="w1kv_sb", tag="w1kv_sb")
            b1q_sb = pp.tile([128, 2], F32, name="b1q_sb", tag="b1q_sb")
            bkv_sb = pp.tile([128, 512], F32, name="bkv_sb", tag="bkv_sb")
            m0_sb = pp.tile([128, BLK], F32, name="m0_sb", tag="m0_sb")
            m1_sb = pp.tile([128, BLK], F32, name="m1_sb", tag="m1_sb")
            id_sb = pp.tile([128, 128], BF16, name="id_sb", tag="id_sb")
            # per-head zero-padded Q^T: head h valid rows (h%2)*64..
            qTp = [pp.tile([128, T], BF16, name=f"qTp{h}", tag=f"qTp{h}") for h in range(4)]
            # packed K^T per head-pair pg: rows = 2 heads x 64 feats
            kT = [pp.tile([128, T], BF16, name=f"kT{g}", tag=f"kT{g}") for g in range(2)]
            # per 128-token chunk: [tok, K(256) | V(256)]
            kv_sb = [pp.tile([128, 512], BF16, name=f"kv{t}", tag=f"kv{t}") for t in range(NTC)]
            oT_sb = [pp.tile([128, T], BF16, name=f"oT{g}", tag=f"oT{g}") for g in range(2)]

            with tc.tile_pool(name="xt", bufs=1) as xp:
                xt = [xp.tile([128, T], BF16, name=f"xt{i}", tag=f"xt{i}") for i in range(ND)]
                # A single DMA transfer runs on ONE of the 16 DMA engines at
                # ~22 GB/s, so per-transfer latency (not queue trigger cost)
                # gates the head.  Split the first chunks into small pieces
                # across FOUR issuing queues so W1's first contraction chunk
                # is resident ~3.5us in instead of ~11us.
                HT = T // 2

                def xrow(i):
                    return xT[128 * i:128 * (i + 1), :]

                # first chunk pieces first so W1's d=0 matmuls can start ASAP
                nc.sync.dma_start(out=w1q_sb[:, 0:FH], in_=w1q[0:128, :])
                nc.scalar.dma_start(out=xt[0][:, 0:512], in_=xrow(0)[:, 0:512])
                nc.gpsimd.dma_start(out=w1kv_sb[:, 0:2 * FH], in_=w1kv[0:128, :])
                nc.scalar.dma_start(out=xt[0][:, 512:HT], in_=xrow(0)[:, 512:HT])
                nc.scalar.dma_start(out=b1q_sb[:], in_=b1q.rearrange("(c p) -> p c", p=128))
                # chunks 1-7: x first-half in 2 pieces (sync+scalar), w1q on
                # sync, w1kv on gpsimd
                for i in range(1, ND):
                    nc.sync.dma_start(out=xt[i][:, 0:512], in_=xrow(i)[:, 0:512])
                    nc.scalar.dma_start(out=xt[i][:, 512:HT], in_=xrow(i)[:, 512:HT])
                    nc.sync.dma_start(
                        out=w1q_sb[:, i * FH:(i + 1) * FH],
                        in_=w1q[128 * i:128 * (i + 1), :])
                    nc.gpsimd.dma_start(
                        out=w1kv_sb[:, i * 2 * FH:(i + 1) * 2 * FH],
                        in_=w1kv[128 * i:128 * (i + 1), :])
                # constants needed by copybacks / phase C
                nc.gpsimd.dma_start(out=bkv_sb[:], in_=b1kv.unsqueeze(0).broadcast_to([128, 512]))
                nc.gpsimd.dma_start(out=m0_sb[:], in_=mask0[:])
                nc.gpsimd.dma_start(out=m1_sb[:], in_=mask1[:])
                nc.gpsimd.dma_start(out=id_sb[:], in_=ident[:])
                # second halves, two pieces each, on sync/gpsimd (the scalar
                # queue is needed for W1 copybacks by now)
                for i in range(ND):
                    nc.sync.dma_start(out=xt[i][:, HT:HT + 512], in_=xrow(i)[:, HT:HT + 512])
                    nc.gpsimd.dma_start(out=xt[i][:, HT + 512:T], in_=xrow(i)[:, HT + 512:T])

                # ---- phase B: projections ----------------------------------
                # Waves of concurrent PSUM groups; within a wave the
                # contraction chunk d is the OUTER loop so the (in-order) PE
                # stream can run each chunk's matmuls as soon as that chunk
                # lands.
                _pools = [psA, psU, psO, psA, psU, psO, psA, psA]

                _ptag = {id(psA): "pa", id(psU): "pu", id(psO): "po"}

                def run_wave(groups, pools=None):
                    pl = pools if pools is not None else _pools
                    tiles = []
                    for gi, _ in enumerate(groups):
                        pool = pl[gi]
                        tiles.append(pool.tile([128, 512], F32, name=f"pw{gi}",
                                               tag=_ptag[id(pool)]))
                    for d in range(ND):
                        for gi, (lf, rf, _) in enumerate(groups):
                            nc.tensor.matmul(
                                tiles[gi][:], lf(d), rf(d),
                                start=(d == 0), stop=(d == ND - 1))
                    for gi, (_, _, cb) in enumerate(groups):
                        cb(tiles[gi])

                def q_group(ft, qt):
                    qsl = slice(qt * 512, (qt + 1) * 512)

                    def cb(pt):
                        # split the packed [128,512] result into the two
                        # per-head padded tiles (both on ACT; DVE is loaded)
                        h0, h1 = 2 * ft, 2 * ft + 1
                        nc.scalar.activation(
                            qTp[h0][0:64, qsl], pt[0:64, :],
                            AF.Identity, bias=b1q_sb[0:64, ft:ft + 1])
                        nc.scalar.activation(
                            qTp[h1][64:128, qsl], pt[64:128, :],
                            AF.Identity, bias=b1q_sb[64:128, ft:ft + 1])
                    return (
                        lambda d: w1q_sb[:, d * FH + ft * 128: d * FH + (ft + 1) * 128],
                        lambda d: xt[d][:, qt * 512:(qt + 1) * 512],
                        cb)

                def kv_group(tcn):
                    def cb(pt):
                        nc.vector.tensor_tensor(
                            kv_sb[tcn][:], pt[:], bkv_sb[:], mybir.AluOpType.add)
                    return (
                        lambda d: xt[d][:, tcn * 128:(tcn + 1) * 128],
                        lambda d: w1kv_sb[:, d * 2 * FH:(d + 1) * 2 * FH],
                        cb)

                def transpose_quads(tc4s):
                    # one quad = 4 PE transposes of kv K chunks into one
                    # [128,512] PSUM tile; tc4s lists (pg, tc4) quads where
                    # tc4 covers token chunks 4*tc4 .. 4*tc4+3
                    pools = [psU, psU, psO, psO]
                    for qi, (pg, tc4) in enumerate(tc4s):
                        pool = pools[qi % 4]
                        # transpose output dtype must match its input (bf16)
                        qt_tile = pool.tile([128, 512], BF16, name="tq",
                                            tag=_ptag[id(pool)])
                        for k in range(4):
                            tcn = 4 * tc4 + k
                            nc.tensor.matmul(
                                qt_tile[:, k * 128:(k + 1) * 128],
                                kv_sb[tcn][:, pg * 128:(pg + 1) * 128],
                                id_sb[:], is_transpose=True,
                                start=True, stop=True, skip_group_check=(k > 0))
                        dst = kT[pg][:, tc4 * 512:(tc4 + 1) * 512]
                        if qi % 2 == 0:
                            nc.scalar.activation(dst, qt_tile[:], AF.Identity)
                        else:
                            nc.vector.tensor_copy(dst, qt_tile[:])

                # W1: Q(qt0,1) + KV(tcn0-3) — first-half tokens only
                run_wave([q_group(0, 0), q_group(0, 1), q_group(1, 0), q_group(1, 1),
                          kv_group(0), kv_group(1), kv_group(2), kv_group(3)])
                # W2: KV(tcn4-7) + 16 transposes (tcn0-7)
                run_wave([kv_group(4), kv_group(5), kv_group(6), kv_group(7)],
                         pools=[psA, psA, psA, psA])
                transpose_quads([(0, 0), (1, 0), (0, 1), (1, 1)])
                # zero the never-written halves of qTp; on the gpsimd queue
                # (idle by now), needed only by phase C's score matmuls
                for h in range(4):
                    par = h % 2
                    nc.gpsimd.memset(qTp[h][(1 - par) * 64:(2 - par) * 64, :], 0.0)
                # W3: Q(qt2,3) + KV(tcn8-11) — second half
                run_wave([q_group(0, 2), q_group(0, 3), q_group(1, 2), q_group(1, 3),
                          kv_group(8), kv_group(9), kv_group(10), kv_group(11)])
                # W4: KV(tcn12-15) + 16 transposes (tcn8-15)
                run_wave([kv_group(12), kv_group(13), kv_group(14), kv_group(15)],
                         pools=[psA, psA, psA, psA])
                transpose_quads([(0, 2), (1, 2), (0, 3), (1, 3)])

            # ---- late pool: state tiles + W2 (reuses x^T space) ------------
            with tc.tile_pool(name="late", bufs=1) as lp:
                # one state tile per head pair: diagonal 64x64 blocks hold the
                # two heads' running K^T V; the off-diagonal blocks accumulate
                # cross-head garbage that the Q@S matmuls never touch (M=64
                # column slices + zero-padded qTp rows)
                spad = [lp.tile([128, 128], BF16, name=f"spad{g}", tag=f"spad{g}") for g in range(2)]
                w2_sb = lp.tile([128, 2 * D], BF16, name="w2_sb", tag="w2_sb")
                # manually-rotated a1 ring: the left half of each slot is the
                # always-zero region of the chunk-1 scores; memset it once and
                # let the per-block mask multiply touch only the tril half.
                a1ring = [lp.tile([128, 2 * BLK], BF16, name=f"a1r{i}", tag=f"a1r{i}")
                          for i in range(4)]
                for i in range(4):
                    for par in range(2):
                        nc.gpsimd.memset(a1ring[i][:, par * BLK: par * BLK + 128], 0.0)
                for g in range(2):
                    nc.gpsimd.memset(spad[g][:], 0.0)
                nc.sync.dma_start(
                    out=w2_sb.rearrange("p (c f) -> p c f", c=2),
                    in_=w2.rearrange("(c p) f -> p c f", p=128))

                # ---- phase C: chunked causal attention + interleaved D -----
                # Two-stage software pipeline: block m's scores are emitted
                # before block m-1's O-accumulation chains, so the in-order PE
                # stream always has independent matmuls while the DVE applies
                # causal masks for the previous block.
                ablk = {}

                def scores_block(m):
                    qsl = slice(m * BLK, (m + 1) * BLK)
                    qsl2 = slice(m * BLK + 128, (m + 1) * BLK)
                    for pg in range(2):
                        a0 = lp.tile([128, 2 * BLK], BF16, name="a0", tag="a0", bufs=4)
                        a1 = a1ring[(2 * m + pg) % 4]
                        ablk[(m, pg)] = (a0, a1)
                        for par in range(2):
                            h = 2 * pg + par
                            asl = slice(par * BLK, (par + 1) * BLK)
                            pA = psA.tile([128, 2 * BLK], F32, name="pA", tag="pa")
                            nc.tensor.matmul(
                                pA[:, 0:BLK], kT[pg][:, (2 * m) * 128:(2 * m + 1) * 128],
                                qTp[h][:, qsl], start=True, stop=True)
                            # chunk 2m+1 only sees the last 128 qtoks (N=128)
                            nc.tensor.matmul(
                                pA[:, BLK:BLK + 128], kT[pg][:, (2 * m + 1) * 128:(2 * m + 2) * 128],
                                qTp[h][:, qsl2], start=True, stop=True, skip_group_check=True)
                            nc.vector.tensor_tensor(a0[:, asl], pA[:, 0:BLK], m0_sb[:], mybir.AluOpType.mult)
                            # only the tril half: the left 128 cols stay zero
                            # (GPSIMD cannot read PSUM, so this stays on DVE)
                            nc.vector.tensor_tensor(
                                a1[:, par * BLK + 128: (par + 1) * BLK],
                                pA[:, BLK:BLK + 128], m1_sb[:, 128:BLK],
                                mybir.AluOpType.mult)

                def chains_block(m):
                    qsl = slice(m * BLK, (m + 1) * BLK)
                    for pg in range(2):
                        a0, a1 = ablk.pop((m, pg))
                        # AV uses per-head M=64 stationary V slices so the
                        # accumulated pO rows are clean/packed: one copyback
                        # per head pair instead of two strided halves; the
                        # Q@S terms use M=64 column slices of the pair state.
                        pO = psO.tile([128, BLK], F32, name="pO", tag="po")
                        # all skip_group_check: the interp's zero-region
                        # bookkeeping cannot represent partition-offset PSUM
                        # writes (real ordering is enforced by tile deps; the
                        # hardware start bits are still per-instruction)
                        for par in range(2):
                            vsl = slice(FH + pg * 128 + par * 64, FH + pg * 128 + (par + 1) * 64)
                            hr = slice(par * 64, (par + 1) * 64)
                            asl = slice(par * BLK, (par + 1) * BLK)
                            nc.tensor.matmul(
                                pO[hr, :], kv_sb[2 * m][:, vsl], a0[:, asl],
                                start=True, stop=False, skip_group_check=True)
                            nc.tensor.matmul(
                                pO[hr, :], kv_sb[2 * m + 1][:, vsl], a1[:, asl],
                                start=False, stop=False, skip_group_check=True)
                        nc.tensor.matmul(
                            pO[0:64, :], spad[pg][:, 0:64], qTp[2 * pg][:, qsl],
                            start=False, stop=True, skip_group_check=True)
                        nc.tensor.matmul(
                            pO[64:128, :], spad[pg][:, 64:128], qTp[2 * pg + 1][:, qsl],
                            start=False, stop=True, skip_group_check=True)
                        nc.scalar.activation(oT_sb[pg][:, qsl], pO[:], AF.Identity)

                    for pg in range(2):
                        # S update restricted to this head pair: out [128,128]
                        # whose two diagonal 64x64 blocks are the per-head
                        # K^T V increments (off-diagonal garbage is harmless)
                        vpg = slice(FH + pg * 128, FH + (pg + 1) * 128)
                        pU = psU.tile([128, 128], F32, name="pU", tag="pu")
                        nc.tensor.matmul(
                            pU[:], kv_sb[2 * m][:, pg * 128:(pg + 1) * 128],
                            kv_sb[2 * m][:, vpg], start=True, stop=False)
                        nc.tensor.matmul(
                            pU[:], kv_sb[2 * m + 1][:, pg * 128:(pg + 1) * 128],
                            kv_sb[2 * m + 1][:, vpg], start=False, stop=True)
                        nc.vector.tensor_tensor(
                            spad[pg][:], pU[:], spad[pg][:], mybir.AluOpType.add)

                def proj_tile(qt, half):
                    # two dout chunks (one dc pair) per copyback + DMA
                    tsl = slice(qt * 512, (qt + 1) * 512)
                    base = 0 if half == 0 else ND // 2
                    for pr in range(2):          # pairs within the half
                        dc0 = base + 2 * pr
                        pf = []
                        for j, dc in enumerate((dc0, dc0 + 1)):
                            pool = [psA, psU, psO][(dc + qt) % 3]
                            p = pool.tile([128, 512], F32, name="pf",
                                          tag=_ptag[id(pool)])
                            pf.append(p)
                            for g2 in range(2):
                                nc.tensor.matmul(
                                    p[:],
                                    w2_sb[:, g2 * D + dc * 128: g2 * D + (dc + 1) * 128],
                                    oT_sb[g2][:, tsl],
                                    start=(g2 == 0), stop=(g2 == 1))
                        fs = lp.tile([128, 2, 512], BF16, name="fs", tag="fs", bufs=6)
                        if pr == 0:
                            nc.vector.tensor_copy(fs[:, 0, :], pf[0][:])
                            nc.vector.tensor_copy(fs[:, 1, :], pf[1][:])
                        else:
                            nc.scalar.activation(fs[:, 0, :], pf[0][:], AF.Identity)
                            nc.scalar.activation(fs[:, 1, :], pf[1][:], AF.Identity)
                        dma_eng = nc.gpsimd if pr == 0 else nc.sync
                        dma_eng.dma_start(
                            out=out[dc0 * 128:(dc0 + 2) * 128, tsl].rearrange(
                                "(c p) t -> p c t", p=128),
                            in_=fs[:])

                def proj_tile256(tcn, split_dma=False):
                    # split_dma: final block — one 64 KB transfer per dout
                    # chunk so the tail drain waits on short transfers
                    tsl = slice(tcn * 128, (tcn + 2) * 128)  # 256 tokens
                    for pr in range(4):
                        dc0 = 2 * pr
                        pf = []
                        for j, dc in enumerate((dc0, dc0 + 1)):
                            pool = [psA, psU, psO][dc % 3]
                            p = pool.tile([128, 512], F32, name="pf2",
                                          tag=_ptag[id(pool)])
                            pf.append(p)
                            for g2 in range(2):
                                nc.tensor.matmul(
                                    p[:, 0:BLK],
                                    w2_sb[:, g2 * D + dc * 128: g2 * D + (dc + 1) * 128],
                                    oT_sb[g2][:, tsl],
                                    start=(g2 == 0), stop=(g2 == 1))
                        fs = lp.tile([128, 2, BLK], BF16, name="fs2", tag="fs2", bufs=8)
                        if pr % 2 == 0:
                            nc.vector.tensor_copy(fs[:, 0, :], pf[0][:, 0:BLK])
                            nc.vector.tensor_copy(fs[:, 1, :], pf[1][:, 0:BLK])
                        else:
                            nc.scalar.activation(fs[:, 0, :], pf[0][:, 0:BLK], AF.Identity)
                            nc.scalar.activation(fs[:, 1, :], pf[1][:, 0:BLK], AF.Identity)
                        if split_dma:
                            for j, dc in enumerate((dc0, dc0 + 1)):
                                dma_eng = nc.gpsimd if (2 * pr + j) % 2 == 0 else nc.sync
                                dma_eng.dma_start(
                                    out=out[dc * 128:(dc + 1) * 128, tsl],
                                    in_=fs[:, j, :])
                        else:
                            dma_eng = nc.gpsimd if pr % 2 == 0 else nc.sync
                            dma_eng.dma_start(
                                out=out[dc0 * 128:(dc0 + 2) * 128, tsl].rearrange(
                                    "(c p) t -> p c t", p=128),
                                in_=fs[:])

                # proj tiles are emitted one block after the copybacks that
                # produce their oT inputs; the last two blocks go out at
                # 256-token granularity to shrink the serial tail.
                dplan = {2: (0, 0), 3: (0, 1), 4: (1, 0), 5: (1, 1), 6: (2, 0), 7: (2, 1)}
                scores_block(0)
                for m in range(1, NBLK):
                    scores_block(m)
                    chains_block(m - 1)
                    if m in dplan:
                        proj_tile(*dplan[m])
                    if m == NBLK - 1:
                        proj_tile256(12)   # block 6 (tokens 1536:1792)
                chains_block(NBLK - 1)
                proj_tile256(14, split_dma=True)   # block 7 (tokens 1792:2048)

    nc.compile()
    return nc


_NC = None


def _get_nc():
    global _NC
    if _NC is None:
        _NC = _build()
    return _NC


def make_core_inputs(x, W1, b1, W2, b2):
    """Shard full inputs into the 8 per-core input dicts."""
    x = np.asarray(x, dtype=np.float32)
    W1 = np.asarray(W1, dtype=np.float32)
    b1 = np.asarray(b1, dtype=np.float32)
    W2 = np.asarray(W2, dtype=np.float32)

    p = np.arange(128)[:, None]
    f = np.arange(BLK)[None, :]
    mask0 = (f >= p).astype(np.float32)
    mask1 = (f >= p + 128).astype(np.float32)
    ident = np.eye(128, dtype=np.float32).astype(BF)

    in_maps = []
    for c in range(8):
        b = c // 4
        g = c % 4
        ksl = slice(g * FH, (g + 1) * FH)
        qsl = slice(D + g * FH, D + (g + 1) * FH)
        vsl = slice(2 * D + g * FH, 2 * D + (g + 1) * FH)
        in_maps.append({
            "xT": np.ascontiguousarray(x[b].T).astype(BF),
            "w1q": np.ascontiguousarray(W1[:, qsl]).astype(BF),
            "w1kv": np.ascontiguousarray(
                np.concatenate([W1[:, ksl], W1[:, vsl]], axis=1)).astype(BF),
            "b1q": np.ascontiguousarray(b1[qsl]),
            "b1kv": np.ascontiguousarray(np.concatenate([b1[ksl], b1[vsl]])),
            "w2": np.ascontiguousarray(W2[ksl, :]).astype(BF),
            "mask0": mask0,
            "mask1": mask1,
            "ident": ident,
        })
    return in_maps


def kernel(x, W1, b1, W2, b2):
    nc = _get_nc()
    in_maps = make_core_inputs(x, W1, b1, W2, b2)
    kwargs = {}
    if TRACE:
        kwargs = {"trace": True, "tmpdir": TRACE_DIR}
    res = run_bass_kernel_spmd(nc, in_maps, list(range(8)), **kwargs)
    LAST_RESULTS[0] = res
    b2 = np.asarray(b2, dtype=np.float32)
    out = np.zeros((B, T, D), np.float32)
    for c in range(8):
        out[c // 4] += res.results[c]["out"].astype(np.float32).T
    out += b2[None, None, :]
    return out


# revision 6
# speedup vs baseline: 1.5608x; 1.0120x over previous
"""Trainium2 Bass kernel for causal softmax-free multi-head attention (retention).

Reference computation (per batch b):
    kqv = x @ W1 + b1 ; k, q, v = split(kqv, 3)   [split order k, q, v]
    per head h (dh = 64):  attn = tril(q_h @ k_h^T) ; o_h = attn @ v_h
    out = concat_h(o_h) @ W2 + b2

Sharding: 8 cores = 2 batches x 4 head-groups (4 heads each). Each core
computes its batch's projections restricted to its heads' weight columns,
the attention for its 4 heads, and a partial output projection
(out_local @ W2[rows of its heads]). Host sums the 4 partials per batch.

v2 vs v1:
  - all matmul operands are bf16 (PSUM accumulation stays f32): halves DMA
    traffic in (x/W1/W2 stream as bf16) and out (bf16 partials), and halves
    DVE element time. Max rel err vs f32 reference ~6e-3 (gate 2e-2).
  - K is projected ONCE ([tok, feat] layout inside the KV wave); the
    [feat, tok] layout needed by the score matmuls comes from PE transposes
    (32 x [128,128] via identity), replacing the v1 duplicate K projection
    (32768 PE rows -> 4096).
  - scores select heads by zero-padding Q (qTp[h], memset halves) instead
    of zero-padding K, so the transposed K stays packed.
  - zero fills via engine memset, not DMA (kills the zer input stream).
  - output projection spread earlier across phase C; the last two blocks
    are emitted at 256-token granularity so the tail after the final
    chains is only 2 small proj groups; out-DMAs pair two dout chunks
    per transfer ([128, 2, 256/512] APs) to halve trigger count.

Algorithm: chunked linear attention. tril(QK^T)V is computed per 256-token
block as  O = Q @ S + tril_block(Q K_blk^T) V_blk, with the running state
S = K^T V accumulated over previous blocks ([64,64] per head).

Hardware constraints honored:
  - matmul tiles: K=128 contraction, M=128 stationary, N>=256 moving
    (N=128 only for the PE transposes, which are exact data movement).
  - DMA cannot touch PSUM: every matmul result is copied out via DVE/ACT.
"""

import numpy as np
import ml_dtypes

import concourse.bacc as bacc
import concourse.mybir as mybir
import concourse.tile as tile
from concourse.bass_utils import run_bass_kernel_spmd

F32 = mybir.dt.float32
BF16 = mybir.dt.bfloat16
AF = mybir.ActivationFunctionType
BF = ml_dtypes.bfloat16

B, T, D = 2, 2048, 1024
H, DH = 16, 64
HPC = 4           # heads per core
FH = HPC * DH     # 256 features per core per tensor
BLK = 256         # state-update block (2 x 128-token chunks)
NBLK = T // BLK   # 8
NTC = T // 128    # 16 token chunks
ND = D // 128     # 8 contraction chunks

# set False if mixed-dtype (f32 PSUM + bf16 SBUF) tensor_tensor misbehaves
MIXED_TT = True

TRACE = False
TRACE_DIR = None
LAST_RESULTS = [None]


def _build():
    nc = bacc.Bacc("TRN2", target_bir_lowering=False, debug=False, num_devices=8)

    xT = nc.dram_tensor("xT", [D, T], BF16, kind="ExternalInput").ap()
    w1q = nc.dram_tensor("w1q", [D, FH], BF16, kind="ExternalInput").ap()
    w1kv = nc.dram_tensor("w1kv", [D, 2 * FH], BF16, kind="ExternalInput").ap()
    b1q = nc.dram_tensor("b1q", [FH], F32, kind="ExternalInput").ap()
    b1kv = nc.dram_tensor("b1kv", [2 * FH], F32, kind="ExternalInput").ap()
    w2 = nc.dram_tensor("w2", [FH, D], BF16, kind="ExternalInput").ap()
    mask0 = nc.dram_tensor("mask0", [128, BLK], F32, kind="ExternalInput").ap()
    mask1 = nc.dram_tensor("mask1", [128, BLK], F32, kind="ExternalInput").ap()
    ident = nc.dram_tensor("ident", [128, 128], BF16, kind="ExternalInput").ap()
    out = nc.dram_tensor("out", [D, T], BF16, kind="ExternalOutput").ap()

    with tile.TileContext(nc) as tc:
        with (
            tc.tile_pool(name="persist", bufs=1) as pp,
            tc.tile_pool(name="work", bufs=3) as wp,
            tc.tile_pool(name="psA", bufs=4, space="PSUM") as psA,
            tc.tile_pool(name="psO", bufs=2, space="PSUM") as psO,
            tc.tile_pool(name="psU", bufs=2, space="PSUM") as psU,
        ):
            # ---- persistent SBUF tiles -------------------------------------
            w1q_sb = pp.tile([128, ND * FH], BF16, name="w1q_sb", tag="w1q_sb")
            w1kv_sb = pp.tile([128, ND * 2 * FH], BF16, name="w1kv_sb", tag="w1kv_sb")
            b1q_sb = pp.tile([128, 2], F32, name="b1q_sb", tag="b1q_sb")
            bkv_sb = pp.tile([128, 512], F32, name="bkv_sb", tag="bkv_sb")
            m0_sb = pp.tile([128, BLK], F32, name="m0_sb", tag="m0_sb")
            m1_sb = pp.tile([128, BLK], F32, name="m1_sb", tag="m1_sb")
            id_sb = pp.tile([128, 128], BF16, name="id_sb", tag="id_sb")
            # per-head zero-padded Q^T: head h valid rows (h%2)*64..
            qTp = [pp.tile([128, T], BF16, name=f"qTp{h}", tag=f"qTp{h}") for h in range(4)]
            # packed K^T per head-pair pg: rows = 2 heads x 64 feats
            kT = [pp.tile([128, T], BF16, name=f"kT{g}", tag=f"kT{g}") for g in range(2)]
            # per 128-token chunk: [tok, K(256) | V(256)]
            kv_sb = [pp.tile([128, 512], BF16, name=f"kv{t}", tag=f"kv{t}") for t in range(NTC)]
            oT_sb = [pp.tile([128, T], BF16, name=f"oT{g}", tag=f"oT{g}") for g in range(2)]

            with tc.tile_pool(name="xt", bufs=1) as xp:
                xt = [xp.tile([128, T], BF16, name=f"xt{i}", tag=f"xt{i}") for i in range(ND)]
                # A single DMA transfer runs on ONE of the 16 DMA engines at
                # ~22 GB/s, so per-transfer latency (not queue trigger cost)
                # gates the head.  Split the first chunks into small pieces
                # across FOUR issuing queues so W1's first contraction chunk
                # is resident ~3.5us in instead of ~11us.
                HT = T // 2

                def xrow(i):
                    return xT[128 * i:128 * (i + 1), :]

                # first chunk pieces first, small (32-64 KB) so W1's d=0
                # matmuls can start ASAP
                nc.sync.dma_start(out=w1q_sb[:, 0:FH], in_=w1q[0:128, :])
                nc.scalar.dma_start(out=xt[0][:, 0:256], in_=xrow(0)[:, 0:256])
                nc.gpsimd.dma_start(out=xt[0][:, 256:512], in_=xrow(0)[:, 256:512])
                nc.scalar.dma_start(out=xt[0][:, 512:768], in_=xrow(0)[:, 512:768])
                nc.gpsimd.dma_start(out=xt[0][:, 768:HT], in_=xrow(0)[:, 768:HT])
                nc.gpsimd.dma_start(out=w1kv_sb[:, 0:FH], in_=w1kv[0:128, 0:FH])
                nc.scalar.dma_start(out=w1kv_sb[:, FH:2 * FH], in_=w1kv[0:128, FH:2 * FH])
                nc.scalar.dma_start(out=b1q_sb[:], in_=b1q.rearrange("(c p) -> p c", p=128))
                # chunks 1-7: x first-half in 2 pieces (sync+scalar), w1q on
                # sync, w1kv on gpsimd
                for i in range(1, ND):
                    nc.sync.dma_start(out=xt[i][:, 0:512], in_=xrow(i)[:, 0:512])
                    nc.scalar.dma_start(out=xt[i][:, 512:HT], in_=xrow(i)[:, 512:HT])
                    nc.sync.dma_start(
                        out=w1q_sb[:, i * FH:(i + 1) * FH],
                        in_=w1q[128 * i:128 * (i + 1), :])
                    nc.gpsimd.dma_start(
                        out=w1kv_sb[:, i * 2 * FH:(i + 1) * 2 * FH],
                        in_=w1kv[128 * i:128 * (i + 1), :])
                # constants needed by copybacks / phase C
                nc.gpsimd.dma_start(out=bkv_sb[:], in_=b1kv.unsqueeze(0).broadcast_to([128, 512]))
                nc.gpsimd.dma_start(out=m0_sb[:], in_=mask0[:])
                nc.gpsimd.dma_start(out=m1_sb[:], in_=mask1[:])
                nc.gpsimd.dma_start(out=id_sb[:], in_=ident[:])
                # second halves, two pieces each, spread over scalar/gpsimd
                # (the sync queue still has 14 first-half triggers pending;
                # these must be issued promptly or W3/W4 stall on them)
                for i in range(ND):
                    nc.scalar.dma_start(out=xt[i][:, HT:HT + 512], in_=xrow(i)[:, HT:HT + 512])
                    nc.gpsimd.dma_start(out=xt[i][:, HT + 512:T], in_=xrow(i)[:, HT + 512:T])

                # ---- phase B: projections ----------------------------------
                # Waves of concurrent PSUM groups; within a wave the
                # contraction chunk d is the OUTER loop so the (in-order) PE
                # stream can run each chunk's matmuls as soon as that chunk
                # lands.
                _pools = [psA, psU, psO, psA, psU, psO, psA, psA]

                _ptag = {id(psA): "pa", id(psU): "pu", id(psO): "po"}

                def run_wave(groups, pools=None):
                    pl = pools if pools is not None else _pools
                    tiles = []
                    for gi, _ in enumerate(groups):
                        pool = pl[gi]
                        tiles.append(pool.tile([128, 512], F32, name=f"pw{gi}",
                                               tag=_ptag[id(pool)]))
                    for d in range(ND):
                        for gi, (lf, rf, _) in enumerate(groups):
                            nc.tensor.matmul(
                                tiles[gi][:], lf(d), rf(d),
                                start=(d == 0), stop=(d == ND - 1))
                    for gi, (_, _, cb) in enumerate(groups):
                        cb(tiles[gi])

                def q_group(ft, qt):
                    qsl = slice(qt * 512, (qt + 1) * 512)

                    def cb(pt):
                        # split the packed [128,512] result into the two
                        # per-head padded tiles (both on ACT; DVE is loaded)
                        h0, h1 = 2 * ft, 2 * ft + 1
                        nc.scalar.activation(
                            qTp[h0][0:64, qsl], pt[0:64, :],
                            AF.Identity, bias=b1q_sb[0:64, ft:ft + 1])
                        nc.scalar.activation(
                            qTp[h1][64:128, qsl], pt[64:128, :],
                            AF.Identity, bias=b1q_sb[64:128, ft:ft + 1])
                    return (
                        lambda d: w1q_sb[:, d * FH + ft * 128: d * FH + (ft + 1) * 128],
                        lambda d: xt[d][:, qt * 512:(qt + 1) * 512],
                        cb)

                def kv_group(tcn):
                    def cb(pt):
                        nc.vector.tensor_tensor(
                            kv_sb[tcn][:], pt[:], bkv_sb[:], mybir.AluOpType.add)
                    return (
                        lambda d: xt[d][:, tcn * 128:(tcn + 1) * 128],
                        lambda d: w1kv_sb[:, d * 2 * FH:(d + 1) * 2 * FH],
                        cb)

                def transpose_quads(tc4s):
                    # one quad = 4 PE transposes of kv K chunks into one
                    # [128,512] PSUM tile; tc4s lists (pg, tc4) quads where
                    # tc4 covers token chunks 4*tc4 .. 4*tc4+3
                    pools = [psU, psU, psO, psO]
                    for qi, (pg, tc4) in enumerate(tc4s):
                        pool = pools[qi % 4]
                        # transpose output dtype must match its input (bf16)
                        qt_tile = pool.tile([128, 512], BF16, name="tq",
                                            tag=_ptag[id(pool)])
                        for k in range(4):
                            tcn = 4 * tc4 + k
                            nc.tensor.matmul(
                                qt_tile[:, k * 128:(k + 1) * 128],
                                kv_sb[tcn][:, pg * 128:(pg + 1) * 128],
                                id_sb[:], is_transpose=True,
                                start=True, stop=True, skip_group_check=(k > 0))
                        dst = kT[pg][:, tc4 * 512:(tc4 + 1) * 512]
                        if qi % 2 == 0:
                            nc.scalar.activation(dst, qt_tile[:], AF.Identity)
                        else:
                            nc.vector.tensor_copy(dst, qt_tile[:])

                # W1: Q(qt0,1) + KV(tcn0-3) — first-half tokens only
                run_wave([q_group(0, 0), q_group(0, 1), q_group(1, 0), q_group(1, 1),
                          kv_group(0), kv_group(1), kv_group(2), kv_group(3)])
                # W2: KV(tcn4-7) + 16 transposes (tcn0-7)
                run_wave([kv_group(4), kv_group(5), kv_group(6), kv_group(7)],
                         pools=[psA, psA, psA, psA])
                transpose_quads([(0, 0), (1, 0), (0, 1), (1, 1)])
                # zero the never-written halves of qTp; on the gpsimd queue
                # (idle by now), needed only by phase C's score matmuls
                for h in range(4):
                    par = h % 2
                    nc.gpsimd.memset(qTp[h][(1 - par) * 64:(2 - par) * 64, :], 0.0)
                # W3: Q(qt2,3) + KV(tcn8-11) — second half
                run_wave([q_group(0, 2), q_group(0, 3), q_group(1, 2), q_group(1, 3),
                          kv_group(8), kv_group(9), kv_group(10), kv_group(11)])
                # W4: KV(tcn12-15) + 16 transposes (tcn8-15)
                run_wave([kv_group(12), kv_group(13), kv_group(14), kv_group(15)],
                         pools=[psA, psA, psA, psA])
                transpose_quads([(0, 2), (1, 2), (0, 3), (1, 3)])

            # ---- late pool: state tiles + W2 (reuses x^T space) ------------
            with tc.tile_pool(name="late", bufs=1) as lp:
                # one state tile per head pair: diagonal 64x64 blocks hold the
                # two heads' running K^T V; the off-diagonal blocks accumulate
                # cross-head garbage that the Q@S matmuls never touch (M=64
                # column slices + zero-padded qTp rows)
                spad = [lp.tile([128, 128], BF16, name=f"spad{g}", tag=f"spad{g}") for g in range(2)]
                w2_sb = lp.tile([128, 2 * D], BF16, name="w2_sb", tag="w2_sb")
                # manually-rotated a1 ring: the left half of each slot is the
                # always-zero region of the chunk-1 scores; memset it once and
                # let the per-block mask multiply touch only the tril half.
                a1ring = [lp.tile([128, 2 * BLK], BF16, name=f"a1r{i}", tag=f"a1r{i}")
                          for i in range(4)]
                for i in range(4):
                    for par in range(2):
                        nc.gpsimd.memset(a1ring[i][:, par * BLK: par * BLK + 128], 0.0)
                for g in range(2):
                    nc.gpsimd.memset(spad[g][:], 0.0)
                nc.sync.dma_start(
                    out=w2_sb.rearrange("p (c f) -> p c f", c=2),
                    in_=w2.rearrange("(c p) f -> p c f", p=128))

                # ---- phase C: chunked causal attention + interleaved D -----
                # Two-stage software pipeline: block m's scores are emitted
                # before block m-1's O-accumulation chains, so the in-order PE
                # stream always has independent matmuls while the DVE applies
                # causal masks for the previous block.
                ablk = {}

                def scores_block(m):
                    qsl = slice(m * BLK, (m + 1) * BLK)
                    qsl2 = slice(m * BLK + 128, (m + 1) * BLK)
                    for pg in range(2):
                        a0 = lp.tile([128, 2 * BLK], BF16, name="a0", tag="a0", bufs=4)
                        a1 = a1ring[(2 * m + pg) % 4]
                        ablk[(m, pg)] = (a0, a1)
                        for par in range(2):
                            h = 2 * pg + par
                            asl = slice(par * BLK, (par + 1) * BLK)
                            pA = psA.tile([128, 2 * BLK], F32, name="pA", tag="pa")
                            nc.tensor.matmul(
                                pA[:, 0:BLK], kT[pg][:, (2 * m) * 128:(2 * m + 1) * 128],
                                qTp[h][:, qsl], start=True, stop=True)
                            # chunk 2m+1 only sees the last 128 qtoks (N=128)
                            nc.tensor.matmul(
                                pA[:, BLK:BLK + 128], kT[pg][:, (2 * m + 1) * 128:(2 * m + 2) * 128],
                                qTp[h][:, qsl2], start=True, stop=True, skip_group_check=True)
                            nc.vector.tensor_tensor(a0[:, asl], pA[:, 0:BLK], m0_sb[:], mybir.AluOpType.mult)
                            # only the tril half: the left 128 cols stay zero
                            # (GPSIMD cannot read PSUM, so this stays on DVE)
                            nc.vector.tensor_tensor(
                                a1[:, par * BLK + 128: (par + 1) * BLK],
                                pA[:, BLK:BLK + 128], m1_sb[:, 128:BLK],
                                mybir.AluOpType.mult)

                def chains_block(m):
                    qsl = slice(m * BLK, (m + 1) * BLK)
                    for pg in range(2):
                        a0, a1 = ablk.pop((m, pg))
                        # AV uses per-head M=64 stationary V slices so the
                        # accumulated pO rows are clean/packed: one copyback
                        # per head pair instead of two strided halves; the
                        # Q@S terms use M=64 column slices of the pair state.
                        pO = psO.tile([128, BLK], F32, name="pO", tag="po")
                        # all skip_group_check: the interp's zero-region
                        # bookkeeping cannot represent partition-offset PSUM
                        # writes (real ordering is enforced by tile deps; the
                        # hardware start bits are still per-instruction)
                        for par in range(2):
                            vsl = slice(FH + pg * 128 + par * 64, FH + pg * 128 + (par + 1) * 64)
                            hr = slice(par * 64, (par + 1) * 64)
                            asl = slice(par * BLK, (par + 1) * BLK)
                            nc.tensor.matmul(
                                pO[hr, :], kv_sb[2 * m][:, vsl], a0[:, asl],
                                start=True, stop=False, skip_group_check=True)
                            nc.tensor.matmul(
                                pO[hr, :], kv_sb[2 * m + 1][:, vsl], a1[:, asl],
                                start=False, stop=False, skip_group_check=True)
                        nc.tensor.matmul(
                            pO[0:64, :], spad[pg][:, 0:64], qTp[2 * pg][:, qsl],
                            start=False, stop=True, skip_group_check=True)
                        nc.tensor.matmul(
                            pO[64:128, :], spad[pg][:, 64:128], qTp[2 * pg + 1][:, qsl],
                            start=False, stop=True, skip_group_check=True)
                        nc.scalar.activation(oT_sb[pg][:, qsl], pO[:], AF.Identity)

                    for pg in range(2):
                        # S update restricted to this head pair: out [128,128]
                        # whose two diagonal 64x64 blocks are the per-head
                        # K^T V increments (off-diagonal garbage is harmless)
                        vpg = slice(FH + pg * 128, FH + (pg + 1) * 128)
                        pU = psU.tile([128, 128], F32, name="pU", tag="pu")
                        nc.tensor.matmul(
                            pU[:], kv_sb[2 * m][:, pg * 128:(pg + 1) * 128],
                            kv_sb[2 * m][:, vpg], start=True, stop=False)
                        nc.tensor.matmul(
                            pU[:], kv_sb[2 * m + 1][:, pg * 128:(pg + 1) * 128],
                            kv_sb[2 * m + 1][:, vpg], start=False, stop=True)
                        nc.vector.tensor_tensor(
                            spad[pg][:], pU[:], spad[pg][:], mybir.AluOpType.add)

                def proj_tile(qt, half):
                    # two dout chunks (one dc pair) per copyback + DMA
                    tsl = slice(qt * 512, (qt + 1) * 512)
                    base = 0 if half == 0 else ND // 2
                    for pr in range(2):          # pairs within the half
                        dc0 = base + 2 * pr
                        pf = []
                        for j, dc in enumerate((dc0, dc0 + 1)):
                            pool = [psA, psU, psO][(dc + qt) % 3]
                            p = pool.tile([128, 512], F32, name="pf",
                                          tag=_ptag[id(pool)])
                            pf.append(p)
                            for g2 in range(2):
                                nc.tensor.matmul(
                                    p[:],
                                    w2_sb[:, g2 * D + dc * 128: g2 * D + (dc + 1) * 128],
                                    oT_sb[g2][:, tsl],
                                    start=(g2 == 0), stop=(g2 == 1))
                        fs = lp.tile([128, 2, 512], BF16, name="fs", tag="fs", bufs=6)
                        if pr == 0:
                            nc.vector.tensor_copy(fs[:, 0, :], pf[0][:])
                            nc.vector.tensor_copy(fs[:, 1, :], pf[1][:])
                        else:
                            nc.scalar.activation(fs[:, 0, :], pf[0][:], AF.Identity)
                            nc.scalar.activation(fs[:, 1, :], pf[1][:], AF.Identity)
                        dma_eng = nc.gpsimd if pr == 0 else nc.sync
                        dma_eng.dma_start(
                            out=out[dc0 * 128:(dc0 + 2) * 128, tsl].rearrange(
                                "(c p) t -> p c t", p=128),
                            in_=fs[:])

                def proj_tile256(tcn, split_dma=False):
                    # split_dma: final block — one 64 KB transfer per dout
                    # chunk so the tail drain waits on short transfers
                    tsl = slice(tcn * 128, (tcn + 2) * 128)  # 256 tokens
                    for pr in range(4):
                        dc0 = 2 * pr
                        pf = []
                        for j, dc in enumerate((dc0, dc0 + 1)):
                            pool = [psA, psU, psO][dc % 3]
                            p = pool.tile([128, 512], F32, name="pf2",
                                          tag=_ptag[id(pool)])
                            pf.append(p)
                            for g2 in range(2):
                                nc.tensor.matmul(
                                    p[:, 0:BLK],
                                    w2_sb[:, g2 * D + dc * 128: g2 * D + (dc + 1) * 128],
                                    oT_sb[g2][:, tsl],
                                    start=(g2 == 0), stop=(g2 == 1))
                        fs = lp.tile([128, 2, BLK], BF16, name="fs2", tag="fs2", bufs=8)
                        if pr % 2 == 0:
                            nc.vector.tensor_copy(fs[:, 0, :], pf[0][:, 0:BLK])
                            nc.vector.tensor_copy(fs[:, 1, :], pf[1][:, 0:BLK])
                        else:
                            nc.scalar.activation(fs[:, 0, :], pf[0][:, 0:BLK], AF.Identity)
                            nc.scalar.activation(fs[:, 1, :], pf[1][:, 0:BLK], AF.Identity)
                        if split_dma:
                            for j, dc in enumerate((dc0, dc0 + 1)):
                                dma_eng = [nc.gpsimd, nc.sync, nc.scalar][(2 * pr + j) % 3]
                                dma_eng.dma_start(
                                    out=out[dc * 128:(dc + 1) * 128, tsl],
                                    in_=fs[:, j, :])
                        else:
                            dma_eng = nc.gpsimd if pr % 2 == 0 else nc.sync
                            dma_eng.dma_start(
                                out=out[dc0 * 128:(dc0 + 2) * 128, tsl].rearrange(
                                    "(c p) t -> p c t", p=128),
                                in_=fs[:])

                # proj tiles are emitted one block after the copybacks that
                # produce their oT inputs; the last two blocks go out at
                # 256-token granularity to shrink the serial tail.
                dplan = {2: (0, 0), 3: (0, 1), 4: (1, 0), 5: (1, 1), 6: (2, 0), 7: (2, 1)}
                scores_block(0)
                for m in range(1, NBLK):
                    scores_block(m)
                    chains_block(m - 1)
                    if m in dplan:
                        proj_tile(*dplan[m])
                    if m == NBLK - 1:
                        proj_tile256(12)   # block 6 (tokens 1536:1792)
                chains_block(NBLK - 1)
                proj_tile256(14, split_dma=True)   # block 7 (tokens 1792:2048)

    nc.compile()
    return nc


_NC = None


def _get_nc():
    global _NC
    if _NC is None:
        _NC = _build()
    return _NC


def make_core_inputs(x, W1, b1, W2, b2):
    """Shard full inputs into the 8 per-core input dicts."""
    x = np.asarray(x, dtype=np.float32)
    W1 = np.asarray(W1, dtype=np.float32)
    b1 = np.asarray(b1, dtype=np.float32)
    W2 = np.asarray(W2, dtype=np.float32)

    p = np.arange(128)[:, None]
    f = np.arange(BLK)[None, :]
    mask0 = (f >= p).astype(np.float32)
    mask1 = (f >= p + 128).astype(np.float32)
    ident = np.eye(128, dtype=np.float32).astype(BF)

    in_maps = []
    for c in range(8):
        b = c // 4
        g = c % 4
        ksl = slice(g * FH, (g + 1) * FH)
        qsl = slice(D + g * FH, D + (g + 1) * FH)
        vsl = slice(2 * D + g * FH, 2 * D + (g + 1) * FH)
        in_maps.append({
            "xT": np.ascontiguousarray(x[b].T).astype(BF),
            "w1q": np.ascontiguousarray(W1[:, qsl]).astype(BF),
            "w1kv": np.ascontiguousarray(
                np.concatenate([W1[:, ksl], W1[:, vsl]], axis=1)).astype(BF),
            "b1q": np.ascontiguousarray(b1[qsl]),
            "b1kv": np.ascontiguousarray(np.concatenate([b1[ksl], b1[vsl]])),
            "w2": np.ascontiguousarray(W2[ksl, :]).astype(BF),
            "mask0": mask0,
            "mask1": mask1,
            "ident": ident,
        })
    return in_maps


def kernel(x, W1, b1, W2, b2):
    nc = _get_nc()
    in_maps = make_core_inputs(x, W1, b1, W2, b2)
    kwargs = {}
    if TRACE:
        kwargs = {"trace": True, "tmpdir": TRACE_DIR}
    res = run_bass_kernel_spmd(nc, in_maps, list(range(8)), **kwargs)
    LAST_RESULTS[0] = res
    b2 = np.asarray(b2, dtype=np.float32)
    out = np.zeros((B, T, D), np.float32)
    for c in range(8):
        out[c // 4] += res.results[c]["out"].astype(np.float32).T
    out += b2[None, None, :]
    return out
